# revision 41
# baseline (speedup 1.0000x reference)
"""Trainium2 Bass kernel for decoder-only GQA attention (tensor-parallel x8).

Problem (hardcoded): B=1, S=2048, HID=4096, H=32 q-heads, KVH=8 kv-heads,
D=128, KV_LEN=2048, seq_position=0, batch_position=0, causal mask.

Sharding: tensor-parallel over the 8 kv heads.  Core c owns kv head c and
q heads 4c..4c+3.  Wq/Wk/Wv sharded along their output (head) dim, Wo along
its input dim.  Each core computes a partial o_proj output [2048, 4096];
a per-chunk ReduceScatter sums the partials and leaves row-shard slices
that the host reassembles (the "gather" half of the hinted all-reduce is
done by the host-side unshard).

Device-side dataflow is entirely "transposed" to keep every matmul
transpose-free:
  hiddenT [hid, s] -> QT/KT [d, s] (RoPE applied in the transposed layout
  via a partition-rotation DMA and sign-folded sin), VT -> V via PE
  transpose, scoresT [kv, q] = K @ QT, softmax along the partition (kv)
  axis with the denominator computed by a ones-column matmul, ctxT [d, q]
  = V.T @ expST, o [q, hid] = ctxT.T @ WoT.

Matmuls run in float32r (fp32 with reduced mantissa, 4x the fp32 rate,
~1e-4 matmul error); everything else is fp32.
"""

import math

import numpy as np

import concourse.bacc as bacc
import concourse.mybir as mybir
import concourse.tile as tile
from concourse.bass_utils import run_bass_kernel_spmd
from concourse.masks import make_identity

S = 2048
HID = 4096
H = 32
KVH = 8
D = 128
G = H // KVH  # q heads per core
KV = 2048
N_CORES = 8
QCH = 512  # q-rows per chunk
NCH = S // QCH  # 4 chunks
HT = HID // 128  # 32 h-tiles
NEG = -1.0e9

F32 = mybir.dt.float32
F32R = mybir.dt.float32r

# Set by test.py to collect HW timing/profiles.
TRACE = False
LAST_RESULT = None

_compiled = None


def _build():
    nc = bacc.Bacc("TRN2", target_bir_lowering=False, num_devices=N_CORES)

    hT = nc.declare_dram_parameter("hT", [HID, S], F32R, isOutput=False)
    wqT = nc.declare_dram_parameter("wqT", [HID, G * D], F32R, isOutput=False)
    wkT = nc.declare_dram_parameter("wkT", [HID, D], F32R, isOutput=False)
    wvT = nc.declare_dram_parameter("wvT", [HID, D], F32R, isOutput=False)
    woT = nc.declare_dram_parameter("woT", [G * D, HID], F32R, isOutput=False)
    cosT = nc.declare_dram_parameter("cosT", [D, S], F32, isOutput=False)
    sinT = nc.declare_dram_parameter("sinT", [D, S], F32, isOutput=False)
    bias4 = nc.declare_dram_parameter("bias4", [4, 128, QCH], F32, isOutput=False)

    kT_out = nc.declare_dram_parameter("kT_out", [D, S], F32, isOutput=True)
    v_out = nc.declare_dram_parameter("v_out", [KV, D], F32, isOutput=True)
    o_out = nc.declare_dram_parameter("o_out", [S // N_CORES, HID], F32, isOutput=True)

    from contextlib import ExitStack

    with tile.TileContext(nc) as tc, ExitStack() as ctx_stack:
        _body(
            nc, tc, ctx_stack, hT, wqT, wkT, wvT, woT, cosT, sinT, bias4,
            kT_out, v_out, o_out,
        )
    nc.compile()
    return nc


def _body(nc, tc, ctx_stack, hT, wqT, wkT, wvT, woT, cosT, sinT, bias4, kT_out, v_out, o_out):
    ec = ctx_stack.enter_context
    persist = ec(tc.tile_pool(name="persist", bufs=1))
    wbig = ec(tc.tile_pool(name="wbig", bufs=1))
    stream = ec(tc.tile_pool(name="stream", bufs=4))
    tmp = ec(tc.tile_pool(name="tmp", bufs=4))
    es_pool = ec(tc.tile_pool(name="es", bufs=3))
    ctx_pool = ec(tc.tile_pool(name="ctx", bufs=4))
    osb_pool = ec(tc.tile_pool(name="osb", bufs=2))
    dram = ec(tc.tile_pool(name="dram", bufs=1, space="DRAM"))

    # --- persistent SBUF ---
    ident = persist.tile([128, 128], F32)
    make_identity(nc, ident[:])
    ones_f32 = persist.tile([128, 1], F32)
    nc.vector.memset(ones_f32[:], 1.0)
    ones_col = persist.tile([128, 1], F32R)
    nc.vector.tensor_copy(ones_col[:], ones_f32[:])
    onesr_f32 = persist.tile([1, 128], F32)
    nc.vector.memset(onesr_f32[:], 1.0)
    ones_row = persist.tile([1, 128], F32R)
    nc.vector.tensor_copy(ones_row[:], onesr_f32[:])

    wk_sb = persist.tile([128, HT * D], F32R)  # [p, (t d)]
    nc.gpsimd.dma_start(
        out=wk_sb[:].rearrange("p (t d) -> p t d", d=D),
        in_=wkT.rearrange("(t p) d -> p t d", p=128),
    )
    wv_sb = persist.tile([128, HT * D], F32R)
    nc.gpsimd.dma_start(
        out=wv_sb[:].rearrange("p (t d) -> p t d", d=D),
        in_=wvT.rearrange("(t p) d -> p t d", p=128),
    )
    bias_sb = persist.tile([128, 4 * QCH], F32)
    nc.gpsimd.dma_start(
        out=bias_sb[:].rearrange("p (t n) -> p t n", n=QCH),
        in_=bias4.rearrange("t p n -> p t n"),
    )

    qT_sb = persist.tile([128, G * S], F32R)  # per head: [:, h*S + q]
    kT_sb = persist.tile([128, S], F32R)
    v_sb = persist.tile([128, KV], F32R)  # per kv tile: [:, kv*128:+128] = V block

    # wq is only needed during the projection phase, wo afterwards; they share
    # one 8MB slot.
    # per-h-tile slice DMAs so the t=0 matmuls don't wait for the whole 8MB
    wq_sb = wbig.tile([128, HT * G * D], F32R, tag="w")  # [p, (t m)]
    for t in range(HT):
        nc.gpsimd.dma_start(
            out=wq_sb[:, t * G * D : (t + 1) * G * D],
            in_=wqT[t * 128 : (t + 1) * 128, :],
        )

    o_part = dram.tile([S, HID], F32)
    # RS pieces: big early pieces overlap compute; small tail pieces cut the
    # exposed latency of the final collective.
    pieces = [(0, 512), (512, 512), (1024, 512), (1536, 256), (1792, 256)]
    o_shards = [
        dram.tile([n // N_CORES, HID], F32, tag=f"osh{i}", name=f"o_shard{i}")
        for i, (r0, n) in enumerate(pieces)
    ]

    # ---------------- Phase A: QKV projection + RoPE + V transpose ----------
    with tc.tile_pool(name="psA", bufs=1, space="PSUM") as psA:
        for j in range(NCH):
            q0 = QCH * j
            cos_sb = stream.tile([128, QCH], F32, tag="cs", bufs=2)
            sin_sb = stream.tile([128, QCH], F32, tag="cs", bufs=2)
            nc.gpsimd.dma_start(out=cos_sb[:], in_=cosT[:, q0 : q0 + QCH])
            nc.gpsimd.dma_start(out=sin_sb[:], in_=sinT[:, q0 : q0 + QCH])

            ps = [
                psA.tile([128, QCH], F32, tag="proj", name=f"proj{j}_{m}", bufs=6)
                for m in range(6)
            ]
            for t in range(HT):
                ht = stream.tile([128, QCH], F32R, tag="ht", bufs=6)
                nc.sync.dma_start(
                    out=ht[:], in_=hT[t * 128 : (t + 1) * 128, q0 : q0 + QCH]
                )
                fl = dict(start=(t == 0), stop=(t == HT - 1), skip_group_check=True)
                for m in range(G):
                    nc.tensor.matmul(
                        ps[m][:],
                        wq_sb[:, t * G * D + m * 128 : t * G * D + (m + 1) * 128],
                        ht[:],
                        **fl,
                    )
                nc.tensor.matmul(
                    ps[4][:], wk_sb[:, t * 128 : (t + 1) * 128], ht[:], **fl
                )
                nc.tensor.matmul(
                    ps[5][:], wv_sb[:, t * 128 : (t + 1) * 128], ht[:], **fl
                )

            # RoPE for the 4 q heads and k: out = ps*cos + rot(ps)*sin_eff
            for m in range(5):
                src = ps[m] if m < G else ps[4]
                if m < G:
                    dst = qT_sb[:, m * S + q0 : m * S + q0 + QCH]
                else:
                    dst = kT_sb[:, q0 : q0 + QCH]
                qraw = tmp.tile([128, QCH], F32, tag="scratch")
                nc.scalar.copy(qraw[:], src[:])
                perm = tmp.tile([128, QCH], F32, tag="scratch")
                nc.scalar.dma_start(out=perm[0:64, :], in_=qraw[64:128, :])
                nc.scalar.dma_start(out=perm[64:128, :], in_=qraw[0:64, :])
                # in-place: qraw *= cos, perm *= sin_eff, dst = qraw + perm
                nc.vector.tensor_mul(qraw[:], qraw[:], cos_sb[:])
                nc.vector.tensor_mul(perm[:], perm[:], sin_sb[:])
                nc.vector.tensor_add(dst, qraw[:], perm[:])
            nc.scalar.dma_start(
                out=kT_out[:, q0 : q0 + QCH],
                in_=kT_sb[:, q0 : q0 + QCH].bitcast(F32),
            )

            # V: copy PSUM -> SBUF, PE-transpose 128x128 blocks into v_sb.
            vt = tmp.tile([128, QCH], F32, tag="scratch")
            nc.scalar.copy(vt[:], ps[5][:])
            for b in range(QCH // 128):
                kvi = 4 * j + b
                pst = psA.tile([128, 128], F32, tag="tr", bufs=2)
                nc.tensor.transpose(pst[:], vt[:, b * 128 : (b + 1) * 128], ident[:])
                nc.vector.tensor_copy(v_sb[:, kvi * 128 : (kvi + 1) * 128], pst[:])
                nc.scalar.dma_start(
                    out=v_out[kvi * 128 : (kvi + 1) * 128, :],
                    in_=v_sb[:, kvi * 128 : (kvi + 1) * 128].bitcast(F32),
                )

    # wo replaces wq in the shared slot; per-dh slices so o_proj dh=0 starts early
    wo_sb = wbig.tile([128, G * HID], F32R, tag="w")  # [p, (dh n)]
    for dh in range(G):
        nc.gpsimd.dma_start(
            out=wo_sb[:, dh * HID : (dh + 1) * HID],
            in_=woT[dh * 128 : (dh + 1) * 128, :],
        )

    # ---------------- Phase B: attention + o_proj + ReduceScatter -----------
    with tc.tile_pool(name="psB", bufs=1, space="PSUM") as psB:
        for j in range(NCH):
            q0 = QCH * j
            nkv = (q0 + QCH) // 128  # causal: kv tiles 0..nkv-1
            ctxs = []
            for h in range(G):
                ctx_ps = psB.tile([128, QCH], F32, tag="ctx", bufs=2)
                den_ps = psB.tile([1, QCH], F32, tag="den", bufs=2)
                for kv in range(nkv):
                    s_ps = psB.tile([128, QCH], F32, tag="s", bufs=4)
                    nc.tensor.matmul(
                        s_ps[:],
                        kT_sb[:, kv * 128 : (kv + 1) * 128],
                        qT_sb[:, h * S + q0 : h * S + q0 + QCH],
                        start=True,
                        stop=True,
                    )
                    db = kv - (nkv - 4)
                    if db >= 0:  # diagonal band: apply mask bias
                        nc.vector.tensor_add(
                            s_ps[:], s_ps[:], bias_sb[:, db * QCH : (db + 1) * QCH]
                        )
                    es = es_pool.tile([128, QCH], F32R, tag="es")
                    nc.scalar.activation(
                        es[:], s_ps[:], mybir.ActivationFunctionType.Exp
                    )
                    flk = dict(
                        start=(kv == 0), stop=(kv == nkv - 1), skip_group_check=True
                    )
                    nc.tensor.matmul(
                        ctx_ps[:], v_sb[:, kv * 128 : (kv + 1) * 128], es[:], **flk
                    )
                    nc.tensor.matmul(den_ps[:], ones_col[:], es[:], **flk)
                inv = tmp.tile([1, QCH], F32R, tag="inv", bufs=2)
                with nc.allow_low_precision(reason="f32r softmax denom"):
                    nc.vector.reciprocal(inv[:], den_ps[:])
                bc_ps = psB.tile([128, QCH], F32, tag="s", bufs=4)
                nc.tensor.matmul(
                    bc_ps[:], ones_row[:], inv[:], start=True, stop=True
                )
                ctxc = tmp.tile([128, QCH], F32, tag="scratch")
                nc.scalar.copy(ctxc[:], ctx_ps[:])
                ctx_sb = ctx_pool.tile([128, QCH], F32R, tag="ctx_sb")
                nc.vector.tensor_mul(ctx_sb[:], ctxc[:], bc_ps[:])
                ctxs.append(ctx_sb)

            for qt in range(QCH // 128):
                for nh in range(HID // QCH):
                    o_ps = psB.tile([128, QCH], F32, tag="s", bufs=4)
                    for dh in range(G):
                        nc.tensor.matmul(
                            o_ps[:],
                            ctxs[dh][:, qt * 128 : (qt + 1) * 128],
                            wo_sb[:, dh * HID + nh * QCH : dh * HID + (nh + 1) * QCH],
                            start=(dh == 0),
                            stop=(dh == G - 1),
                            skip_group_check=True,
                        )
                    o_sb = osb_pool.tile([128, QCH], F32, tag="osb")
                    nc.vector.tensor_copy(o_sb[:], o_ps[:])
                    nc.scalar.dma_start(
                        out=o_part[
                            q0 + qt * 128 : q0 + (qt + 1) * 128,
                            nh * QCH : (nh + 1) * QCH,
                        ],
                        in_=o_sb[:],
                    )
                # fire the RS for any piece whose rows are now fully written
                row_end = q0 + (qt + 1) * 128
                for i, (r0, n) in enumerate(pieces):
                    if r0 + n == row_end:
                        nc.gpsimd.collective_compute(
                            "ReduceScatter",
                            mybir.AluOpType.add,
                            replica_groups=[list(range(N_CORES))],
                            ins=[o_part[r0 : r0 + n, :]],
                            outs=[o_shards[i][:]],
                        )
                        nc.scalar.dma_start(
                            out=o_out[r0 // N_CORES : (r0 + n) // N_CORES, :],
                            in_=o_shards[i][:],
                        )


def _prep_inputs(hidden_states, attention_mask, cos, sin, Wq, Wk, Wv, Wo):
    h = np.ascontiguousarray(np.asarray(hidden_states, np.float32).reshape(S, HID))
    hT = np.ascontiguousarray(h.T)
    cos2 = np.asarray(cos, np.float32).reshape(S, D)
    sin2 = np.asarray(sin, np.float32).reshape(S, D)
    cosT = np.ascontiguousarray(cos2.T)
    sgn = np.where(np.arange(D) < D // 2, -1.0, 1.0).astype(np.float32)
    sinT = np.ascontiguousarray((sin2 * sgn).T)

    mask2 = np.asarray(attention_mask, np.float32).reshape(S, KV)
    # The kernel hardcodes the causal block structure; verify it holds.
    expect = np.tril(np.ones((S, KV), np.float32))
    if not np.array_equal(mask2, expect):
        raise ValueError("kernel compiled for a causal (tril) attention_mask")
    bias4 = np.empty((4, 128, QCH), np.float32)
    for t in range(4):
        sub = mask2[0:QCH, t * 128 : (t + 1) * 128]  # [q, kv_local]
        bias4[t] = np.where(sub.T > 0.5, 0.0, NEG).astype(np.float32)

    scale = 1.0 / math.sqrt(D)
    Wq = np.asarray(Wq, np.float32)
    Wk = np.asarray(Wk, np.float32)
    Wv = np.asarray(Wv, np.float32)
    Wo = np.asarray(Wo, np.float32)

    in_maps = []
    for c in range(N_CORES):
        wq_c = np.ascontiguousarray((Wq[c * G * D : (c + 1) * G * D, :] * scale).T)
        wk_c = np.ascontiguousarray(Wk[c * D : (c + 1) * D, :].T)
        wv_c = np.ascontiguousarray(Wv[c * D : (c + 1) * D, :].T)
        wo_c = np.ascontiguousarray(Wo[:, c * G * D : (c + 1) * G * D].T)
        in_maps.append(
            dict(
                hT=hT,
                wqT=wq_c,
                wkT=wk_c,
                wvT=wv_c,
                woT=wo_c,
                cosT=cosT,
                sinT=sinT,
                bias4=bias4,
            )
        )
    return in_maps


def kernel(
    hidden_states,
    attention_mask,
    cos,
    sin,
    past_key,
    past_value,
    Wq,
    Wk,
    Wv,
    Wo,
    seq_positions,
    batch_position,
):
    global _compiled, LAST_RESULT
    assert int(np.asarray(seq_positions).reshape(-1)[0]) == 0
    assert int(np.asarray(batch_position)) == 0

    if _compiled is None:
        _compiled = _build()
    nc = _compiled

    in_maps = _prep_inputs(hidden_states, attention_mask, cos, sin, Wq, Wk, Wv, Wo)
    res = run_bass_kernel_spmd(nc, in_maps, list(range(N_CORES)), trace=TRACE)
    LAST_RESULT = res

    pieces = [(0, 512), (512, 512), (1024, 512), (1536, 256), (1792, 256)]
    key_cache = np.empty((1, KVH, KV, D), np.float32)
    value_cache = np.empty((1, KVH, KV, D), np.float32)
    attn_out = np.empty((S, HID), np.float32)
    for c in range(N_CORES):
        r = res.results[c]
        key_cache[0, c] = r["kT_out"].T
        value_cache[0, c] = r["v_out"]
        for r0, n in pieces:
            sh = n // N_CORES
            attn_out[r0 + sh * c : r0 + sh * (c + 1)] = r["o_out"][
                r0 // N_CORES : r0 // N_CORES + sh
            ]
    return attn_out.reshape(1, S, HID), key_cache, value_cache


# revision 43
# speedup vs baseline: 1.0145x; 1.0145x over previous
"""Trainium2 Bass kernel for decoder-only GQA attention (tensor-parallel x8).

Problem (hardcoded): B=1, S=2048, HID=4096, H=32 q-heads, KVH=8 kv-heads,
D=128, KV_LEN=2048, seq_position=0, batch_position=0, causal mask.

Sharding: tensor-parallel over the 8 kv heads.  Core c owns kv head c and
q heads 4c..4c+3.  Wq/Wk/Wv sharded along their output (head) dim, Wo along
its input dim.  Each core computes a partial o_proj output [2048, 4096];
a per-chunk ReduceScatter sums the partials and leaves row-shard slices
that the host reassembles (the "gather" half of the hinted all-reduce is
done by the host-side unshard).

Device-side dataflow is entirely "transposed" to keep every matmul
transpose-free:
  hiddenT [hid, s] -> QT/KT [d, s] (RoPE applied in the transposed layout
  via a partition-rotation DMA and sign-folded sin), VT -> V via PE
  transpose, scoresT [kv, q] = K @ QT, softmax along the partition (kv)
  axis with the denominator computed by a ones-column matmul, ctxT [d, q]
  = V.T @ expST, o [q, hid] = ctxT.T @ WoT.

Matmuls run in float32r (fp32 with reduced mantissa, 4x the fp32 rate,
~1e-4 matmul error); everything else is fp32.
"""

import math

import numpy as np

import concourse.bacc as bacc
import concourse.mybir as mybir
import concourse.tile as tile
from concourse.bass_utils import run_bass_kernel_spmd
from concourse.masks import make_identity

S = 2048
HID = 4096
H = 32
KVH = 8
D = 128
G = H // KVH  # q heads per core
KV = 2048
N_CORES = 8
QCH = 512  # q-rows per chunk
NCH = S // QCH  # 4 chunks
HT = HID // 128  # 32 h-tiles
NEG = -1.0e9

F32 = mybir.dt.float32
F32R = mybir.dt.float32r

# Set by test.py to collect HW timing/profiles.
TRACE = False
LAST_RESULT = None

_compiled = None


def _build():
    nc = bacc.Bacc("TRN2", target_bir_lowering=False, num_devices=N_CORES)

    hT = nc.declare_dram_parameter("hT", [HID, S], F32R, isOutput=False)
    wqT = nc.declare_dram_parameter("wqT", [HID, G * D], F32R, isOutput=False)
    wkT = nc.declare_dram_parameter("wkT", [HID, D], F32R, isOutput=False)
    wvT = nc.declare_dram_parameter("wvT", [HID, D], F32R, isOutput=False)
    woT = nc.declare_dram_parameter("woT", [G * D, HID], F32R, isOutput=False)
    cosT = nc.declare_dram_parameter("cosT", [D, S], F32, isOutput=False)
    sinT = nc.declare_dram_parameter("sinT", [D, S], F32, isOutput=False)
    bias4 = nc.declare_dram_parameter("bias4", [4, 128, QCH], F32, isOutput=False)

    kT_out = nc.declare_dram_parameter("kT_out", [D, S], F32, isOutput=True)
    v_out = nc.declare_dram_parameter("v_out", [KV, D], F32, isOutput=True)
    o_out = nc.declare_dram_parameter("o_out", [S // N_CORES, HID], F32, isOutput=True)

    from contextlib import ExitStack

    with tile.TileContext(nc) as tc, ExitStack() as ctx_stack:
        _body(
            nc, tc, ctx_stack, hT, wqT, wkT, wvT, woT, cosT, sinT, bias4,
            kT_out, v_out, o_out,
        )
    nc.compile()
    return nc


def _body(nc, tc, ctx_stack, hT, wqT, wkT, wvT, woT, cosT, sinT, bias4, kT_out, v_out, o_out):
    ec = ctx_stack.enter_context
    persist = ec(tc.tile_pool(name="persist", bufs=1))
    wbig = ec(tc.tile_pool(name="wbig", bufs=1))
    stream = ec(tc.tile_pool(name="stream", bufs=4))
    tmp = ec(tc.tile_pool(name="tmp", bufs=4))
    es_pool = ec(tc.tile_pool(name="es", bufs=3))
    ctx_pool = ec(tc.tile_pool(name="ctx", bufs=4))
    osb_pool = ec(tc.tile_pool(name="osb", bufs=2))
    dram = ec(tc.tile_pool(name="dram", bufs=1, space="DRAM"))

    # --- persistent SBUF ---
    ident = persist.tile([128, 128], F32)
    make_identity(nc, ident[:])
    ones_f32 = persist.tile([128, 1], F32)
    nc.vector.memset(ones_f32[:], 1.0)
    ones_col = persist.tile([128, 1], F32R)
    nc.vector.tensor_copy(ones_col[:], ones_f32[:])
    onesr_f32 = persist.tile([1, 128], F32)
    nc.vector.memset(onesr_f32[:], 1.0)
    ones_row = persist.tile([1, 128], F32R)
    nc.vector.tensor_copy(ones_row[:], onesr_f32[:])

    wk_sb = persist.tile([128, HT * D], F32R)  # [p, (t d)]
    nc.gpsimd.dma_start(
        out=wk_sb[:].rearrange("p (t d) -> p t d", d=D),
        in_=wkT.rearrange("(t p) d -> p t d", p=128),
    )
    wv_sb = persist.tile([128, HT * D], F32R)
    nc.gpsimd.dma_start(
        out=wv_sb[:].rearrange("p (t d) -> p t d", d=D),
        in_=wvT.rearrange("(t p) d -> p t d", p=128),
    )
    bias_sb = persist.tile([128, 4 * QCH], F32)
    nc.gpsimd.dma_start(
        out=bias_sb[:].rearrange("p (t n) -> p t n", n=QCH),
        in_=bias4.rearrange("t p n -> p t n"),
    )

    qT_sb = persist.tile([128, G * S], F32R)  # per head: [:, h*S + q]
    kT_sb = persist.tile([128, S], F32R)
    v_sb = persist.tile([128, KV], F32R)  # per kv tile: [:, kv*128:+128] = V block

    # wq is only needed during the projection phase, wo afterwards; they share
    # one 8MB slot.
    # per-h-tile slice DMAs so the t=0 matmuls don't wait for the whole 8MB
    wq_sb = wbig.tile([128, HT * G * D], F32R, tag="w")  # [p, (t m)]
    for t in range(HT):
        nc.gpsimd.dma_start(
            out=wq_sb[:, t * G * D : (t + 1) * G * D],
            in_=wqT[t * 128 : (t + 1) * 128, :],
        )

    # RS pieces: big early pieces overlap compute; small tail pieces cut the
    # exposed latency of the final collective.  One DRAM tile per piece —
    # a single big o_part tile would give later stores a false whole-tile
    # WAR dependency on each in-flight ReduceScatter.
    pieces = [(0, 512), (512, 512), (1024, 512), (1536, 256), (1792, 256)]
    o_parts = [
        dram.tile([n, HID], F32, tag=f"opart{i}", name=f"o_part{i}")
        for i, (r0, n) in enumerate(pieces)
    ]
    o_shards = [
        dram.tile([n // N_CORES, HID], F32, tag=f"osh{i}", name=f"o_shard{i}")
        for i, (r0, n) in enumerate(pieces)
    ]

    def piece_of_row(row):
        for i, (r0, n) in enumerate(pieces):
            if r0 <= row < r0 + n:
                return i, r0
        raise AssertionError(row)

    # ---------------- Phase A: QKV projection + RoPE + V transpose ----------
    with tc.tile_pool(name="psA", bufs=1, space="PSUM") as psA:
        for j in range(NCH):
            q0 = QCH * j
            cos_sb = stream.tile([128, QCH], F32, tag="cs", bufs=2)
            sin_sb = stream.tile([128, QCH], F32, tag="cs", bufs=2)
            nc.gpsimd.dma_start(out=cos_sb[:], in_=cosT[:, q0 : q0 + QCH])
            nc.gpsimd.dma_start(out=sin_sb[:], in_=sinT[:, q0 : q0 + QCH])

            ps = [
                psA.tile([128, QCH], F32, tag="proj", name=f"proj{j}_{m}", bufs=6)
                for m in range(6)
            ]
            for t in range(HT):
                ht = stream.tile([128, QCH], F32R, tag="ht", bufs=6)
                nc.sync.dma_start(
                    out=ht[:], in_=hT[t * 128 : (t + 1) * 128, q0 : q0 + QCH]
                )
                fl = dict(start=(t == 0), stop=(t == HT - 1), skip_group_check=True)
                for m in range(G):
                    nc.tensor.matmul(
                        ps[m][:],
                        wq_sb[:, t * G * D + m * 128 : t * G * D + (m + 1) * 128],
                        ht[:],
                        **fl,
                    )
                nc.tensor.matmul(
                    ps[4][:], wk_sb[:, t * 128 : (t + 1) * 128], ht[:], **fl
                )
                nc.tensor.matmul(
                    ps[5][:], wv_sb[:, t * 128 : (t + 1) * 128], ht[:], **fl
                )

            # RoPE for the 4 q heads and k: out = ps*cos + rot(ps)*sin_eff
            for m in range(5):
                src = ps[m] if m < G else ps[4]
                if m < G:
                    dst = qT_sb[:, m * S + q0 : m * S + q0 + QCH]
                else:
                    dst = kT_sb[:, q0 : q0 + QCH]
                qraw = tmp.tile([128, QCH], F32, tag="scratch")
                nc.scalar.copy(qraw[:], src[:])
                perm = tmp.tile([128, QCH], F32, tag="scratch")
                nc.scalar.dma_start(out=perm[0:64, :], in_=qraw[64:128, :])
                nc.scalar.dma_start(out=perm[64:128, :], in_=qraw[0:64, :])
                # in-place: qraw *= cos, perm *= sin_eff, dst = qraw + perm
                nc.vector.tensor_mul(qraw[:], qraw[:], cos_sb[:])
                nc.vector.tensor_mul(perm[:], perm[:], sin_sb[:])
                nc.vector.tensor_add(dst, qraw[:], perm[:])
            nc.scalar.dma_start(
                out=kT_out[:, q0 : q0 + QCH],
                in_=kT_sb[:, q0 : q0 + QCH].bitcast(F32),
            )

            # V: copy PSUM -> SBUF, PE-transpose 128x128 blocks into v_sb.
            vt = tmp.tile([128, QCH], F32, tag="scratch")
            nc.scalar.copy(vt[:], ps[5][:])
            for b in range(QCH // 128):
                kvi = 4 * j + b
                pst = psA.tile([128, 128], F32, tag="tr", bufs=2)
                nc.tensor.transpose(pst[:], vt[:, b * 128 : (b + 1) * 128], ident[:])
                nc.vector.tensor_copy(v_sb[:, kvi * 128 : (kvi + 1) * 128], pst[:])
                nc.scalar.dma_start(
                    out=v_out[kvi * 128 : (kvi + 1) * 128, :],
                    in_=v_sb[:, kvi * 128 : (kvi + 1) * 128].bitcast(F32),
                )

    # wo replaces wq in the shared slot; per-dh slices so o_proj dh=0 starts early
    wo_sb = wbig.tile([128, G * HID], F32R, tag="w")  # [p, (dh n)]
    for dh in range(G):
        nc.gpsimd.dma_start(
            out=wo_sb[:, dh * HID : (dh + 1) * HID],
            in_=woT[dh * 128 : (dh + 1) * 128, :],
        )

    # ---------------- Phase B: attention + o_proj + ReduceScatter -----------
    with tc.tile_pool(name="psB", bufs=1, space="PSUM") as psB:
        for j in range(NCH):
            q0 = QCH * j
            nkv = (q0 + QCH) // 128  # causal: kv tiles 0..nkv-1
            ctxs = []
            for h in range(G):
                ctx_ps = psB.tile([128, QCH], F32, tag="ctx", bufs=2)
                den_ps = psB.tile([1, QCH], F32, tag="den", bufs=2)
                for kv in range(nkv):
                    s_ps = psB.tile([128, QCH], F32, tag="s", bufs=4)
                    nc.tensor.matmul(
                        s_ps[:],
                        kT_sb[:, kv * 128 : (kv + 1) * 128],
                        qT_sb[:, h * S + q0 : h * S + q0 + QCH],
                        start=True,
                        stop=True,
                    )
                    db = kv - (nkv - 4)
                    if db >= 0:  # diagonal band: apply mask bias
                        nc.vector.tensor_add(
                            s_ps[:], s_ps[:], bias_sb[:, db * QCH : (db + 1) * QCH]
                        )
                    es = es_pool.tile([128, QCH], F32R, tag="es")
                    nc.scalar.activation(
                        es[:], s_ps[:], mybir.ActivationFunctionType.Exp
                    )
                    flk = dict(
                        start=(kv == 0), stop=(kv == nkv - 1), skip_group_check=True
                    )
                    nc.tensor.matmul(
                        ctx_ps[:], v_sb[:, kv * 128 : (kv + 1) * 128], es[:], **flk
                    )
                    nc.tensor.matmul(den_ps[:], ones_col[:], es[:], **flk)
                inv = tmp.tile([1, QCH], F32R, tag="inv", bufs=2)
                with nc.allow_low_precision(reason="f32r softmax denom"):
                    nc.vector.reciprocal(inv[:], den_ps[:])
                bc_ps = psB.tile([128, QCH], F32, tag="s", bufs=4)
                nc.tensor.matmul(
                    bc_ps[:], ones_row[:], inv[:], start=True, stop=True
                )
                ctxc = tmp.tile([128, QCH], F32, tag="scratch")
                nc.scalar.copy(ctxc[:], ctx_ps[:])
                ctx_sb = ctx_pool.tile([128, QCH], F32R, tag="ctx_sb")
                nc.vector.tensor_mul(ctx_sb[:], ctxc[:], bc_ps[:])
                ctxs.append(ctx_sb)

            for qt in range(QCH // 128):
                for nh in range(HID // QCH):
                    o_ps = psB.tile([128, QCH], F32, tag="s", bufs=4)
                    for dh in range(G):
                        nc.tensor.matmul(
                            o_ps[:],
                            ctxs[dh][:, qt * 128 : (qt + 1) * 128],
                            wo_sb[:, dh * HID + nh * QCH : dh * HID + (nh + 1) * QCH],
                            start=(dh == 0),
                            stop=(dh == G - 1),
                            skip_group_check=True,
                        )
                    o_sb = osb_pool.tile([128, QCH], F32, tag="osb")
                    nc.vector.tensor_copy(o_sb[:], o_ps[:])
                    row = q0 + qt * 128
                    pi, pr0 = piece_of_row(row)
                    nc.scalar.dma_start(
                        out=o_parts[pi][
                            row - pr0 : row - pr0 + 128, nh * QCH : (nh + 1) * QCH
                        ],
                        in_=o_sb[:],
                    )
                # fire the RS for any piece whose rows are now fully written
                row_end = q0 + (qt + 1) * 128
                for i, (r0, n) in enumerate(pieces):
                    if r0 + n == row_end:
                        nc.gpsimd.collective_compute(
                            "ReduceScatter",
                            mybir.AluOpType.add,
                            replica_groups=[list(range(N_CORES))],
                            ins=[o_parts[i][:]],
                            outs=[o_shards[i][:]],
                        )
                        nc.scalar.dma_start(
                            out=o_out[r0 // N_CORES : (r0 + n) // N_CORES, :],
                            in_=o_shards[i][:],
                        )


def _prep_inputs(hidden_states, attention_mask, cos, sin, Wq, Wk, Wv, Wo):
    h = np.ascontiguousarray(np.asarray(hidden_states, np.float32).reshape(S, HID))
    hT = np.ascontiguousarray(h.T)
    cos2 = np.asarray(cos, np.float32).reshape(S, D)
    sin2 = np.asarray(sin, np.float32).reshape(S, D)
    cosT = np.ascontiguousarray(cos2.T)
    sgn = np.where(np.arange(D) < D // 2, -1.0, 1.0).astype(np.float32)
    sinT = np.ascontiguousarray((sin2 * sgn).T)

    mask2 = np.asarray(attention_mask, np.float32).reshape(S, KV)
    # The kernel hardcodes the causal block structure; verify it holds.
    expect = np.tril(np.ones((S, KV), np.float32))
    if not np.array_equal(mask2, expect):
        raise ValueError("kernel compiled for a causal (tril) attention_mask")
    bias4 = np.empty((4, 128, QCH), np.float32)
    for t in range(4):
        sub = mask2[0:QCH, t * 128 : (t + 1) * 128]  # [q, kv_local]
        bias4[t] = np.where(sub.T > 0.5, 0.0, NEG).astype(np.float32)

    scale = 1.0 / math.sqrt(D)
    Wq = np.asarray(Wq, np.float32)
    Wk = np.asarray(Wk, np.float32)
    Wv = np.asarray(Wv, np.float32)
    Wo = np.asarray(Wo, np.float32)

    in_maps = []
    for c in range(N_CORES):
        wq_c = np.ascontiguousarray((Wq[c * G * D : (c + 1) * G * D, :] * scale).T)
        wk_c = np.ascontiguousarray(Wk[c * D : (c + 1) * D, :].T)
        wv_c = np.ascontiguousarray(Wv[c * D : (c + 1) * D, :].T)
        wo_c = np.ascontiguousarray(Wo[:, c * G * D : (c + 1) * G * D].T)
        in_maps.append(
            dict(
                hT=hT,
                wqT=wq_c,
                wkT=wk_c,
                wvT=wv_c,
                woT=wo_c,
                cosT=cosT,
                sinT=sinT,
                bias4=bias4,
            )
        )
    return in_maps


def kernel(
    hidden_states,
    attention_mask,
    cos,
    sin,
    past_key,
    past_value,
    Wq,
    Wk,
    Wv,
    Wo,
    seq_positions,
    batch_position,
):
    global _compiled, LAST_RESULT
    assert int(np.asarray(seq_positions).reshape(-1)[0]) == 0
    assert int(np.asarray(batch_position)) == 0

    if _compiled is None:
        _compiled = _build()
    nc = _compiled

    in_maps = _prep_inputs(hidden_states, attention_mask, cos, sin, Wq, Wk, Wv, Wo)
    res = run_bass_kernel_spmd(nc, in_maps, list(range(N_CORES)), trace=TRACE)
    LAST_RESULT = res

    pieces = [(0, 512), (512, 512), (1024, 512), (1536, 256), (1792, 256)]
    key_cache = np.empty((1, KVH, KV, D), np.float32)
    value_cache = np.empty((1, KVH, KV, D), np.float32)
    attn_out = np.empty((S, HID), np.float32)
    for c in range(N_CORES):
        r = res.results[c]
        key_cache[0, c] = r["kT_out"].T
        value_cache[0, c] = r["v_out"]
        for r0, n in pieces:
            sh = n // N_CORES
            attn_out[r0 + sh * c : r0 + sh * (c + 1)] = r["o_out"][
                r0 // N_CORES : r0 // N_CORES + sh
            ]
    return attn_out.reshape(1, S, HID), key_cache, value_cache


# revision 44
# speedup vs baseline: 1.1021x; 1.0863x over previous
"""Trainium2 Bass kernel for decoder-only GQA attention (tensor-parallel x8).

Problem (hardcoded): B=1, S=2048, HID=4096, H=32 q-heads, KVH=8 kv-heads,
D=128, KV_LEN=2048, seq_position=0, batch_position=0, causal mask.

Sharding: tensor-parallel over the 8 kv heads.  Core c owns kv head c and
q heads 4c..4c+3.  Wq/Wk/Wv sharded along their output (head) dim, Wo along
its input dim.  Each core computes a partial o_proj output [2048, 4096];
a per-chunk ReduceScatter sums the partials and leaves row-shard slices
that the host reassembles (the "gather" half of the hinted all-reduce is
done by the host-side unshard).

Device-side dataflow is entirely "transposed" to keep every matmul
transpose-free:
  hiddenT [hid, s] -> QT/KT [d, s] (RoPE applied in the transposed layout
  via a partition-rotation DMA and sign-folded sin), VT -> V via PE
  transpose, scoresT [kv, q] = K @ QT, softmax along the partition (kv)
  axis with the denominator computed by a ones-column matmul, ctxT [d, q]
  = V.T @ expST, o [q, hid] = ctxT.T @ WoT.

Matmuls run in float32r (fp32 with reduced mantissa, 4x the fp32 rate,
~1e-4 matmul error); everything else is fp32.
"""

import math

import numpy as np

import concourse.bacc as bacc
import concourse.mybir as mybir
import concourse.tile as tile
from concourse.bass_utils import run_bass_kernel_spmd
from concourse.masks import make_identity

S = 2048
HID = 4096
H = 32
KVH = 8
D = 128
G = H // KVH  # q heads per core
KV = 2048
N_CORES = 8
QCH = 512  # q-rows per chunk
NCH = S // QCH  # 4 chunks
HT = HID // 128  # 32 h-tiles
NEG = -1.0e9

F32 = mybir.dt.float32
F32R = mybir.dt.float32r

# Set by test.py to collect HW timing/profiles.
TRACE = False
LAST_RESULT = None

_compiled = None


def _build():
    nc = bacc.Bacc("TRN2", target_bir_lowering=False, num_devices=N_CORES)

    hT = nc.declare_dram_parameter("hT", [HID, S], F32R, isOutput=False)
    wqT = nc.declare_dram_parameter("wqT", [HID, G * D], F32R, isOutput=False)
    wkT = nc.declare_dram_parameter("wkT", [HID, D], F32R, isOutput=False)
    wvT = nc.declare_dram_parameter("wvT", [HID, D], F32R, isOutput=False)
    woT = nc.declare_dram_parameter("woT", [G * D, HID], F32R, isOutput=False)
    cosT = nc.declare_dram_parameter("cosT", [D, S], F32, isOutput=False)
    sinT = nc.declare_dram_parameter("sinT", [D, S], F32, isOutput=False)
    bias4 = nc.declare_dram_parameter("bias4", [4, 128, QCH], F32, isOutput=False)

    kT_out = nc.declare_dram_parameter("kT_out", [D, S], F32, isOutput=True)
    v_out = nc.declare_dram_parameter("v_out", [KV, D], F32, isOutput=True)
    o_out = nc.declare_dram_parameter("o_out", [S // N_CORES, HID], F32, isOutput=True)

    from contextlib import ExitStack

    with tile.TileContext(nc) as tc, ExitStack() as ctx_stack:
        _body(
            nc, tc, ctx_stack, hT, wqT, wkT, wvT, woT, cosT, sinT, bias4,
            kT_out, v_out, o_out,
        )
    nc.compile()
    return nc


def _body(nc, tc, ctx_stack, hT, wqT, wkT, wvT, woT, cosT, sinT, bias4, kT_out, v_out, o_out):
    ec = ctx_stack.enter_context
    persist = ec(tc.tile_pool(name="persist", bufs=1))
    wbig = ec(tc.tile_pool(name="wbig", bufs=1))
    stream = ec(tc.tile_pool(name="stream", bufs=4))
    tmp = ec(tc.tile_pool(name="tmp", bufs=4))
    es_pool = ec(tc.tile_pool(name="es", bufs=3))
    ctx_pool = ec(tc.tile_pool(name="ctx", bufs=4))
    osb_pool = ec(tc.tile_pool(name="osb", bufs=2))
    dram = ec(tc.tile_pool(name="dram", bufs=1, space="DRAM"))

    # --- persistent SBUF ---
    ident = persist.tile([128, 128], F32)
    make_identity(nc, ident[:])
    ones_f32 = persist.tile([128, 1], F32)
    nc.vector.memset(ones_f32[:], 1.0)
    ones_col = persist.tile([128, 1], F32R)
    nc.vector.tensor_copy(ones_col[:], ones_f32[:])
    onesr_f32 = persist.tile([1, 128], F32)
    nc.vector.memset(onesr_f32[:], 1.0)
    ones_row = persist.tile([1, 128], F32R)
    nc.vector.tensor_copy(ones_row[:], onesr_f32[:])

    wk_sb = persist.tile([128, HT * D], F32R)  # [p, (t d)]
    nc.gpsimd.dma_start(
        out=wk_sb[:].rearrange("p (t d) -> p t d", d=D),
        in_=wkT.rearrange("(t p) d -> p t d", p=128),
    )
    wv_sb = persist.tile([128, HT * D], F32R)
    nc.gpsimd.dma_start(
        out=wv_sb[:].rearrange("p (t d) -> p t d", d=D),
        in_=wvT.rearrange("(t p) d -> p t d", p=128),
    )
    bias_sb = persist.tile([128, 4 * QCH], F32)
    nc.gpsimd.dma_start(
        out=bias_sb[:].rearrange("p (t n) -> p t n", n=QCH),
        in_=bias4.rearrange("t p n -> p t n"),
    )

    qT_sb = persist.tile([128, G * S], F32R)  # per head: [:, h*S + q]
    kT_sb = persist.tile([128, S], F32R)
    v_sb = persist.tile([128, KV], F32R)  # per kv tile: [:, kv*128:+128] = V block

    # wq is only needed during the projection phase, wo afterwards; they share
    # one 8MB slot.
    # per-h-tile slice DMAs so the t=0 matmuls don't wait for the whole 8MB
    wq_sb = wbig.tile([128, HT * G * D], F32R, tag="w")  # [p, (t m)]
    for t in range(HT):
        nc.gpsimd.dma_start(
            out=wq_sb[:, t * G * D : (t + 1) * G * D],
            in_=wqT[t * 128 : (t + 1) * 128, :],
        )

    # RS pieces: big early pieces overlap compute; small tail pieces cut the
    # exposed latency of the final collective.  One DRAM tile per piece —
    # a single big o_part tile would give later stores a false whole-tile
    # WAR dependency on each in-flight ReduceScatter.
    pieces = [(0, 512), (512, 512), (1024, 512), (1536, 256), (1792, 256)]
    o_parts = [
        dram.tile([n, HID], F32, tag=f"opart{i}", name=f"o_part{i}")
        for i, (r0, n) in enumerate(pieces)
    ]
    o_shards = [
        dram.tile([n // N_CORES, HID], F32, tag=f"osh{i}", name=f"o_shard{i}")
        for i, (r0, n) in enumerate(pieces)
    ]

    def piece_of_row(row):
        for i, (r0, n) in enumerate(pieces):
            if r0 <= row < r0 + n:
                return i, r0
        raise AssertionError(row)

    # ---------------- Phase A: QKV projection + RoPE + V transpose ----------
    with tc.tile_pool(name="psA", bufs=1, space="PSUM") as psA:
        for j in range(NCH):
            q0 = QCH * j
            cos_sb = stream.tile([128, QCH], F32, tag="cs", bufs=2)
            sin_sb = stream.tile([128, QCH], F32, tag="cs", bufs=2)
            nc.gpsimd.dma_start(out=cos_sb[:], in_=cosT[:, q0 : q0 + QCH])
            nc.gpsimd.dma_start(out=sin_sb[:], in_=sinT[:, q0 : q0 + QCH])

            ps = [
                psA.tile([128, QCH], F32, tag="proj", name=f"proj{j}_{m}", bufs=6)
                for m in range(6)
            ]
            for t in range(HT):
                ht = stream.tile([128, QCH], F32R, tag="ht", bufs=6)
                nc.sync.dma_start(
                    out=ht[:], in_=hT[t * 128 : (t + 1) * 128, q0 : q0 + QCH]
                )
                fl = dict(start=(t == 0), stop=(t == HT - 1), skip_group_check=True)
                for m in range(G):
                    nc.tensor.matmul(
                        ps[m][:],
                        wq_sb[:, t * G * D + m * 128 : t * G * D + (m + 1) * 128],
                        ht[:],
                        **fl,
                    )
                nc.tensor.matmul(
                    ps[4][:], wk_sb[:, t * 128 : (t + 1) * 128], ht[:], **fl
                )
                nc.tensor.matmul(
                    ps[5][:], wv_sb[:, t * 128 : (t + 1) * 128], ht[:], **fl
                )

            # RoPE for the 4 q heads and k: out = ps*cos + rot(ps)*sin_eff
            for m in range(5):
                src = ps[m] if m < G else ps[4]
                if m < G:
                    dst = qT_sb[:, m * S + q0 : m * S + q0 + QCH]
                else:
                    dst = kT_sb[:, q0 : q0 + QCH]
                qraw = tmp.tile([128, QCH], F32, tag="scratch")
                nc.scalar.copy(qraw[:], src[:])
                perm = tmp.tile([128, QCH], F32, tag="scratch")
                nc.scalar.dma_start(out=perm[0:64, :], in_=qraw[64:128, :])
                nc.scalar.dma_start(out=perm[64:128, :], in_=qraw[0:64, :])
                # in-place: qraw *= cos, perm *= sin_eff, dst = qraw + perm
                nc.vector.tensor_mul(qraw[:], qraw[:], cos_sb[:])
                nc.vector.tensor_mul(perm[:], perm[:], sin_sb[:])
                nc.vector.tensor_add(dst, qraw[:], perm[:])
            nc.scalar.dma_start(
                out=kT_out[:, q0 : q0 + QCH],
                in_=kT_sb[:, q0 : q0 + QCH].bitcast(F32),
            )

            # V: copy PSUM -> SBUF, PE-transpose 128x128 blocks into v_sb.
            vt = tmp.tile([128, QCH], F32, tag="scratch")
            nc.scalar.copy(vt[:], ps[5][:])
            for b in range(QCH // 128):
                kvi = 4 * j + b
                pst = psA.tile([128, 128], F32, tag="tr", bufs=2)
                nc.tensor.transpose(pst[:], vt[:, b * 128 : (b + 1) * 128], ident[:])
                nc.vector.tensor_copy(v_sb[:, kvi * 128 : (kvi + 1) * 128], pst[:])
                nc.scalar.dma_start(
                    out=v_out[kvi * 128 : (kvi + 1) * 128, :],
                    in_=v_sb[:, kvi * 128 : (kvi + 1) * 128].bitcast(F32),
                )

    # wo replaces wq in the shared slot; per-dh slices so o_proj dh=0 starts early
    wo_sb = wbig.tile([128, G * HID], F32R, tag="w")  # [p, (dh n)]
    for dh in range(G):
        nc.gpsimd.dma_start(
            out=wo_sb[:, dh * HID : (dh + 1) * HID],
            in_=woT[dh * 128 : (dh + 1) * 128, :],
        )

    # ---------------- Phase B: attention + o_proj + ReduceScatter -----------
    with tc.tile_pool(name="psB", bufs=1, space="PSUM") as psB:
        for j in range(NCH):
            q0 = QCH * j
            nkv = (q0 + QCH) // 128  # causal: kv tiles 0..nkv-1
            ctxs = []
            for h in range(G):
                ctx_ps = psB.tile([128, QCH], F32, tag="ctx", bufs=2)
                den_ps = psB.tile([1, QCH], F32, tag="den", bufs=2)
                for kv in range(nkv):
                    s_ps = psB.tile([128, QCH], F32, tag="s", bufs=4)
                    nc.tensor.matmul(
                        s_ps[:],
                        kT_sb[:, kv * 128 : (kv + 1) * 128],
                        qT_sb[:, h * S + q0 : h * S + q0 + QCH],
                        start=True,
                        stop=True,
                    )
                    db = kv - (nkv - 4)
                    if db >= 0:  # diagonal band: apply mask bias
                        nc.vector.tensor_add(
                            s_ps[:], s_ps[:], bias_sb[:, db * QCH : (db + 1) * QCH]
                        )
                    es = es_pool.tile([128, QCH], F32R, tag="es")
                    nc.scalar.activation(
                        es[:], s_ps[:], mybir.ActivationFunctionType.Exp
                    )
                    flk = dict(
                        start=(kv == 0), stop=(kv == nkv - 1), skip_group_check=True
                    )
                    nc.tensor.matmul(
                        ctx_ps[:], v_sb[:, kv * 128 : (kv + 1) * 128], es[:], **flk
                    )
                    nc.tensor.matmul(den_ps[:], ones_col[:], es[:], **flk)
                inv = tmp.tile([1, QCH], F32R, tag="inv", bufs=2)
                with nc.allow_low_precision(reason="f32r softmax denom"):
                    nc.vector.reciprocal(inv[:], den_ps[:])
                bc_ps = psB.tile([128, QCH], F32, tag="s", bufs=4)
                nc.tensor.matmul(
                    bc_ps[:], ones_row[:], inv[:], start=True, stop=True
                )
                ctxc = tmp.tile([128, QCH], F32, tag="scratch")
                nc.scalar.copy(ctxc[:], ctx_ps[:])
                ctx_sb = ctx_pool.tile([128, QCH], F32R, tag="ctx_sb")
                nc.vector.tensor_mul(ctx_sb[:], ctxc[:], bc_ps[:])
                ctxs.append(ctx_sb)

            for qt in range(QCH // 128):
                for nh in range(HID // QCH):
                    o_ps = psB.tile([128, QCH], F32, tag="s", bufs=4)
                    for dh in range(G):
                        nc.tensor.matmul(
                            o_ps[:],
                            ctxs[dh][:, qt * 128 : (qt + 1) * 128],
                            wo_sb[:, dh * HID + nh * QCH : dh * HID + (nh + 1) * QCH],
                            start=(dh == 0),
                            stop=(dh == G - 1),
                            skip_group_check=True,
                        )
                    o_sb = osb_pool.tile([128, QCH], F32, tag="osb")
                    nc.vector.tensor_copy(o_sb[:], o_ps[:])
                    row = q0 + qt * 128
                    pi, pr0 = piece_of_row(row)
                    nc.scalar.dma_start(
                        out=o_parts[pi][
                            row - pr0 : row - pr0 + 128, nh * QCH : (nh + 1) * QCH
                        ],
                        in_=o_sb[:],
                    )
                # fire the RS for any piece whose rows are now fully written
                row_end = q0 + (qt + 1) * 128
                for i, (r0, n) in enumerate(pieces):
                    if r0 + n == row_end:
                        nc.gpsimd.collective_compute(
                            "ReduceScatter",
                            mybir.AluOpType.add,
                            replica_groups=[list(range(N_CORES))],
                            ins=[o_parts[i][:]],
                            outs=[o_shards[i][:]],
                        )
                        # sync queue: a store waiting on the RS would
                        # head-of-line-block the scalar queue's o_part stores
                        nc.sync.dma_start(
                            out=o_out[r0 // N_CORES : (r0 + n) // N_CORES, :],
                            in_=o_shards[i][:],
                        )


def _prep_inputs(hidden_states, attention_mask, cos, sin, Wq, Wk, Wv, Wo):
    h = np.ascontiguousarray(np.asarray(hidden_states, np.float32).reshape(S, HID))
    hT = np.ascontiguousarray(h.T)
    cos2 = np.asarray(cos, np.float32).reshape(S, D)
    sin2 = np.asarray(sin, np.float32).reshape(S, D)
    cosT = np.ascontiguousarray(cos2.T)
    sgn = np.where(np.arange(D) < D // 2, -1.0, 1.0).astype(np.float32)
    sinT = np.ascontiguousarray((sin2 * sgn).T)

    mask2 = np.asarray(attention_mask, np.float32).reshape(S, KV)
    # The kernel hardcodes the causal block structure; verify it holds.
    expect = np.tril(np.ones((S, KV), np.float32))
    if not np.array_equal(mask2, expect):
        raise ValueError("kernel compiled for a causal (tril) attention_mask")
    bias4 = np.empty((4, 128, QCH), np.float32)
    for t in range(4):
        sub = mask2[0:QCH, t * 128 : (t + 1) * 128]  # [q, kv_local]
        bias4[t] = np.where(sub.T > 0.5, 0.0, NEG).astype(np.float32)

    scale = 1.0 / math.sqrt(D)
    Wq = np.asarray(Wq, np.float32)
    Wk = np.asarray(Wk, np.float32)
    Wv = np.asarray(Wv, np.float32)
    Wo = np.asarray(Wo, np.float32)

    in_maps = []
    for c in range(N_CORES):
        wq_c = np.ascontiguousarray((Wq[c * G * D : (c + 1) * G * D, :] * scale).T)
        wk_c = np.ascontiguousarray(Wk[c * D : (c + 1) * D, :].T)
        wv_c = np.ascontiguousarray(Wv[c * D : (c + 1) * D, :].T)
        wo_c = np.ascontiguousarray(Wo[:, c * G * D : (c + 1) * G * D].T)
        in_maps.append(
            dict(
                hT=hT,
                wqT=wq_c,
                wkT=wk_c,
                wvT=wv_c,
                woT=wo_c,
                cosT=cosT,
                sinT=sinT,
                bias4=bias4,
            )
        )
    return in_maps


def kernel(
    hidden_states,
    attention_mask,
    cos,
    sin,
    past_key,
    past_value,
    Wq,
    Wk,
    Wv,
    Wo,
    seq_positions,
    batch_position,
):
    global _compiled, LAST_RESULT
    assert int(np.asarray(seq_positions).reshape(-1)[0]) == 0
    assert int(np.asarray(batch_position)) == 0

    if _compiled is None:
        _compiled = _build()
    nc = _compiled

    in_maps = _prep_inputs(hidden_states, attention_mask, cos, sin, Wq, Wk, Wv, Wo)
    res = run_bass_kernel_spmd(nc, in_maps, list(range(N_CORES)), trace=TRACE)
    LAST_RESULT = res

    pieces = [(0, 512), (512, 512), (1024, 512), (1536, 256), (1792, 256)]
    key_cache = np.empty((1, KVH, KV, D), np.float32)
    value_cache = np.empty((1, KVH, KV, D), np.float32)
    attn_out = np.empty((S, HID), np.float32)
    for c in range(N_CORES):
        r = res.results[c]
        key_cache[0, c] = r["kT_out"].T
        value_cache[0, c] = r["v_out"]
        for r0, n in pieces:
            sh = n // N_CORES
            attn_out[r0 + sh * c : r0 + sh * (c + 1)] = r["o_out"][
                r0 // N_CORES : r0 // N_CORES + sh
            ]
    return attn_out.reshape(1, S, HID), key_cache, value_cache


# revision 52
# speedup vs baseline: 1.1281x; 1.0236x over previous
"""Trainium2 Bass kernel for decoder-only GQA attention (tensor-parallel x8).

Problem (hardcoded): B=1, S=2048, HID=4096, H=32 q-heads, KVH=8 kv-heads,
D=128, KV_LEN=2048, seq_position=0, batch_position=0, causal mask.

Sharding: tensor-parallel over the 8 kv heads.  Core c owns kv head c and
q heads 4c..4c+3.  Wq/Wk/Wv sharded along their output (head) dim, Wo along
its input dim.  Each core computes a partial o_proj output [2048, 4096];
a per-chunk ReduceScatter sums the partials and leaves row-shard slices
that the host reassembles (the "gather" half of the hinted all-reduce is
done by the host-side unshard).

Device-side dataflow is entirely "transposed" to keep every matmul
transpose-free:
  hiddenT [hid, s] -> QT/KT [d, s] (RoPE applied in the transposed layout
  via a partition-rotation DMA and sign-folded sin), VT -> V via PE
  transpose, scoresT [kv, q] = K @ QT, softmax along the partition (kv)
  axis with the denominator computed by a ones-column matmul, ctxT [d, q]
  = V.T @ expST, o [q, hid] = ctxT.T @ WoT.

Matmuls run in float32r (fp32 with reduced mantissa, 4x the fp32 rate,
~1e-4 matmul error); everything else is fp32.
"""

import math

import numpy as np

import concourse.bacc as bacc
import concourse.mybir as mybir
import concourse.tile as tile
from concourse.bass_utils import run_bass_kernel_spmd
from concourse.masks import make_identity

S = 2048
HID = 4096
H = 32
KVH = 8
D = 128
G = H // KVH  # q heads per core
KV = 2048
N_CORES = 8
QCH = 512  # q-rows per chunk
NCH = S // QCH  # 4 chunks
HT = HID // 128  # 32 h-tiles
NEG = -1.0e9

F32 = mybir.dt.float32
F32R = mybir.dt.float32r

# Set by test.py to collect HW timing/profiles.
TRACE = False
LAST_RESULT = None

# ReduceScatter pieces (row ranges of attn_out), shared by device and host code
PIECES = [(r0, 256) for r0 in range(0, S, 256)]

_compiled = None


def _build():
    nc = bacc.Bacc("TRN2", target_bir_lowering=False, num_devices=N_CORES)

    hT = nc.declare_dram_parameter("hT", [HID, S], F32R, isOutput=False)
    wqT = nc.declare_dram_parameter("wqT", [HID, G * D], F32R, isOutput=False)
    wkT = nc.declare_dram_parameter("wkT", [HID, D], F32R, isOutput=False)
    wvT = nc.declare_dram_parameter("wvT", [HID, D], F32R, isOutput=False)
    woT = nc.declare_dram_parameter("woT", [G * D, HID], F32R, isOutput=False)
    cosT = nc.declare_dram_parameter("cosT", [D, S], F32, isOutput=False)
    sinT = nc.declare_dram_parameter("sinT", [D, S], F32, isOutput=False)
    bias4 = nc.declare_dram_parameter("bias4", [4, 128, QCH], F32, isOutput=False)

    kT_out = nc.declare_dram_parameter("kT_out", [D, S], F32, isOutput=True)
    v_out = nc.declare_dram_parameter("v_out", [KV, D], F32, isOutput=True)
    o_out = nc.declare_dram_parameter("o_out", [S // N_CORES, HID], F32, isOutput=True)

    from contextlib import ExitStack

    with tile.TileContext(nc) as tc, ExitStack() as ctx_stack:
        _body(
            nc, tc, ctx_stack, hT, wqT, wkT, wvT, woT, cosT, sinT, bias4,
            kT_out, v_out, o_out,
        )
    nc.compile()
    return nc


def _body(nc, tc, ctx_stack, hT, wqT, wkT, wvT, woT, cosT, sinT, bias4, kT_out, v_out, o_out):
    ec = ctx_stack.enter_context
    persist = ec(tc.tile_pool(name="persist", bufs=1))
    wbig = ec(tc.tile_pool(name="wbig", bufs=1))
    stream = ec(tc.tile_pool(name="stream", bufs=4))
    tmp = ec(tc.tile_pool(name="tmp", bufs=4))
    es_pool = ec(tc.tile_pool(name="es", bufs=3))
    ctx_pool = ec(tc.tile_pool(name="ctx", bufs=4))
    osb_pool = ec(tc.tile_pool(name="osb", bufs=2))
    dram = ec(tc.tile_pool(name="dram", bufs=1, space="DRAM"))

    # --- persistent SBUF ---
    ident = persist.tile([128, 128], F32)
    make_identity(nc, ident[:])
    ones_f32 = persist.tile([128, 1], F32)
    nc.vector.memset(ones_f32[:], 1.0)
    ones_col = persist.tile([128, 1], F32R)
    nc.vector.tensor_copy(ones_col[:], ones_f32[:])
    onesr_f32 = persist.tile([1, 128], F32)
    nc.vector.memset(onesr_f32[:], 1.0)
    ones_row = persist.tile([1, 128], F32R)
    nc.vector.tensor_copy(ones_row[:], onesr_f32[:])

    wk_sb = persist.tile([128, HT * D], F32R)  # [p, (t d)]
    wv_sb = persist.tile([128, HT * D], F32R)
    bias_sb = persist.tile([128, 4 * QCH], F32)
    nc.gpsimd.dma_start(
        out=bias_sb[:].rearrange("p (t n) -> p t n", n=QCH),
        in_=bias4.rearrange("t p n -> p t n"),
    )

    qT_sb = persist.tile([128, G * S], F32R)  # per head: [:, h*S + q]
    kT_sb = persist.tile([128, S], F32R)
    v_sb = persist.tile([128, KV], F32R)  # per kv tile: [:, kv*128:+128] = V block

    # wq is only needed during the projection phase, wo afterwards; they share
    # one 8MB slot.  Per-h-tile slice DMAs, interleaved wq/wk/wv in t order,
    # so the t=0 matmuls start as soon as the first three slices land.
    wq_sb = wbig.tile([128, HT * G * D], F32R, tag="w")  # [p, (t m)]
    for t in range(HT):
        nc.gpsimd.dma_start(
            out=wq_sb[:, t * G * D : (t + 1) * G * D],
            in_=wqT[t * 128 : (t + 1) * 128, :],
        )
        nc.gpsimd.dma_start(
            out=wk_sb[:, t * D : (t + 1) * D], in_=wkT[t * 128 : (t + 1) * 128, :]
        )
        nc.gpsimd.dma_start(
            out=wv_sb[:, t * D : (t + 1) * D], in_=wvT[t * 128 : (t + 1) * 128, :]
        )

    # RS pieces: big early pieces overlap compute; small tail pieces cut the
    # exposed latency of the final collective.  One DRAM tile per piece —
    # a single big o_part tile would give later stores a false whole-tile
    # WAR dependency on each in-flight ReduceScatter.
    pieces = PIECES
    o_parts = [
        dram.tile([n, HID], F32, tag=f"opart{i}", name=f"o_part{i}")
        for i, (r0, n) in enumerate(pieces)
    ]
    o_shards = [
        dram.tile([n // N_CORES, HID], F32, tag=f"osh{i}", name=f"o_shard{i}")
        for i, (r0, n) in enumerate(pieces)
    ]

    def piece_of_row(row):
        for i, (r0, n) in enumerate(pieces):
            if r0 <= row < r0 + n:
                return i, r0
        raise AssertionError(row)

    # ---------------- Phase A: QKV projection + RoPE + V transpose ----------
    with tc.tile_pool(name="psA", bufs=1, space="PSUM") as psA:
        for j in range(NCH):
            q0 = QCH * j
            cos_sb = stream.tile([128, QCH], F32, tag="cs", bufs=2)
            sin_sb = stream.tile([128, QCH], F32, tag="cs", bufs=2)
            nc.gpsimd.dma_start(out=cos_sb[:], in_=cosT[:, q0 : q0 + QCH])
            nc.gpsimd.dma_start(out=sin_sb[:], in_=sinT[:, q0 : q0 + QCH])

            ps = [
                psA.tile([128, QCH], F32, tag="proj", name=f"proj{j}_{m}", bufs=6)
                for m in range(6)
            ]
            for t in range(HT):
                ht = stream.tile([128, QCH], F32R, tag="ht", bufs=6)
                nc.sync.dma_start(
                    out=ht[:], in_=hT[t * 128 : (t + 1) * 128, q0 : q0 + QCH]
                )
                fl = dict(start=(t == 0), stop=(t == HT - 1), skip_group_check=True)
                for m in range(G):
                    nc.tensor.matmul(
                        ps[m][:],
                        wq_sb[:, t * G * D + m * 128 : t * G * D + (m + 1) * 128],
                        ht[:],
                        **fl,
                    )
                nc.tensor.matmul(
                    ps[4][:], wk_sb[:, t * 128 : (t + 1) * 128], ht[:], **fl
                )
                nc.tensor.matmul(
                    ps[5][:], wv_sb[:, t * 128 : (t + 1) * 128], ht[:], **fl
                )

            # V first (its ACT copy unblocks the PE transposes that sit next
            # in the PE instruction stream), then RoPE for the 4 q heads + k.
            vt = tmp.tile([128, QCH], F32, tag="scratch")
            nc.scalar.copy(vt[:], ps[5][:])
            for b in range(QCH // 128):
                kvi = 4 * j + b
                pst = psA.tile([128, 128], F32, tag="tr", bufs=2)
                nc.tensor.transpose(pst[:], vt[:, b * 128 : (b + 1) * 128], ident[:])
                nc.vector.tensor_copy(v_sb[:, kvi * 128 : (kvi + 1) * 128], pst[:])
                nc.scalar.dma_start(
                    out=v_out[kvi * 128 : (kvi + 1) * 128, :],
                    in_=v_sb[:, kvi * 128 : (kvi + 1) * 128].bitcast(F32),
                )

            # RoPE for the 4 q heads and k: out = ps*cos + rot(ps)*sin_eff
            for m in range(5):
                src = ps[m] if m < G else ps[4]
                if m < G:
                    dst = qT_sb[:, m * S + q0 : m * S + q0 + QCH]
                else:
                    dst = kT_sb[:, q0 : q0 + QCH]
                qraw = tmp.tile([128, QCH], F32, tag="scratch")
                nc.scalar.copy(qraw[:], src[:])
                perm = tmp.tile([128, QCH], F32, tag="scratch")
                nc.scalar.dma_start(out=perm[0:64, :], in_=qraw[64:128, :])
                nc.scalar.dma_start(out=perm[64:128, :], in_=qraw[0:64, :])
                # in-place: qraw *= cos, perm *= sin_eff, dst = qraw + perm
                nc.vector.tensor_mul(qraw[:], qraw[:], cos_sb[:])
                nc.vector.tensor_mul(perm[:], perm[:], sin_sb[:])
                nc.vector.tensor_add(dst, qraw[:], perm[:])
            nc.scalar.dma_start(
                out=kT_out[:, q0 : q0 + QCH],
                in_=kT_sb[:, q0 : q0 + QCH].bitcast(F32),
            )

    # wo replaces wq in the shared slot; per-dh slices so o_proj dh=0 starts early
    wo_sb = wbig.tile([128, G * HID], F32R, tag="w")  # [p, (dh n)]
    for dh in range(G):
        nc.gpsimd.dma_start(
            out=wo_sb[:, dh * HID : (dh + 1) * HID],
            in_=woT[dh * 128 : (dh + 1) * 128, :],
        )

    # ---------------- Phase B: attention + o_proj + ReduceScatter -----------
    with tc.tile_pool(name="psB", bufs=1, space="PSUM") as psB:
        for j in range(NCH):
            q0 = QCH * j
            nkv = (q0 + QCH) // 128  # causal: kv tiles 0..nkv-1
            ctxs = [None] * G
            pending = None  # (ctx_ps, den_ps, h) awaiting normalize

            def normalize(ctx_ps, den_ps, h):
                # deferred by one head so the bc matmul (which waits on the
                # DVE reciprocal) doesn't stall the in-order PE stream
                inv = tmp.tile([1, QCH], F32R, tag="inv", bufs=2)
                with nc.allow_low_precision(reason="f32r softmax denom"):
                    nc.vector.reciprocal(inv[:], den_ps[:])
                bc_ps = psB.tile([128, QCH], F32, tag="s", bufs=4)
                nc.tensor.matmul(bc_ps[:], ones_row[:], inv[:], start=True, stop=True)
                ctxc = tmp.tile([128, QCH], F32, tag="scratch")
                nc.scalar.copy(ctxc[:], ctx_ps[:])
                ctx_sb = ctx_pool.tile([128, QCH], F32R, tag="ctx_sb")
                nc.vector.tensor_mul(ctx_sb[:], ctxc[:], bc_ps[:])
                ctxs[h] = ctx_sb

            for h in range(G):
                ctx_ps = psB.tile([128, QCH], F32, tag="ctx", bufs=2)
                den_ps = psB.tile([1, QCH], F32, tag="den", bufs=2)
                for kv in range(nkv):
                    s_ps = psB.tile([128, QCH], F32, tag="s", bufs=4)
                    nc.tensor.matmul(
                        s_ps[:],
                        kT_sb[:, kv * 128 : (kv + 1) * 128],
                        qT_sb[:, h * S + q0 : h * S + q0 + QCH],
                        start=True,
                        stop=True,
                    )
                    db = kv - (nkv - 4)
                    if db >= 0:  # diagonal band: apply mask bias
                        nc.vector.tensor_add(
                            s_ps[:], s_ps[:], bias_sb[:, db * QCH : (db + 1) * QCH]
                        )
                    es = es_pool.tile([128, QCH], F32R, tag="es")
                    nc.scalar.activation(
                        es[:], s_ps[:], mybir.ActivationFunctionType.Exp
                    )
                    flk = dict(
                        start=(kv == 0), stop=(kv == nkv - 1), skip_group_check=True
                    )
                    nc.tensor.matmul(
                        ctx_ps[:], v_sb[:, kv * 128 : (kv + 1) * 128], es[:], **flk
                    )
                    nc.tensor.matmul(den_ps[:], ones_col[:], es[:], **flk)
                if pending is not None:
                    normalize(*pending)
                pending = (ctx_ps, den_ps, h)
            normalize(*pending)

            for qt in range(QCH // 128):
                for nh in range(HID // QCH):
                    o_ps = psB.tile([128, QCH], F32, tag="s", bufs=4)
                    for dh in range(G):
                        nc.tensor.matmul(
                            o_ps[:],
                            ctxs[dh][:, qt * 128 : (qt + 1) * 128],
                            wo_sb[:, dh * HID + nh * QCH : dh * HID + (nh + 1) * QCH],
                            start=(dh == 0),
                            stop=(dh == G - 1),
                            skip_group_check=True,
                        )
                    o_sb = osb_pool.tile([128, QCH], F32, tag="osb")
                    nc.vector.tensor_copy(o_sb[:], o_ps[:])
                    row = q0 + qt * 128
                    pi, pr0 = piece_of_row(row)
                    nc.scalar.dma_start(
                        out=o_parts[pi][
                            row - pr0 : row - pr0 + 128, nh * QCH : (nh + 1) * QCH
                        ],
                        in_=o_sb[:],
                    )
                # fire the RS for any piece whose rows are now fully written
                row_end = q0 + (qt + 1) * 128
                for i, (r0, n) in enumerate(pieces):
                    if r0 + n == row_end:
                        nc.gpsimd.collective_compute(
                            "ReduceScatter",
                            mybir.AluOpType.add,
                            replica_groups=[list(range(N_CORES))],
                            ins=[o_parts[i][:]],
                            outs=[o_shards[i][:]],
                        )
                        # sync queue: a store waiting on the RS would
                        # head-of-line-block the scalar queue's o_part stores
                        nc.sync.dma_start(
                            out=o_out[r0 // N_CORES : (r0 + n) // N_CORES, :],
                            in_=o_shards[i][:],
                        )


def _prep_inputs(hidden_states, attention_mask, cos, sin, Wq, Wk, Wv, Wo):
    h = np.ascontiguousarray(np.asarray(hidden_states, np.float32).reshape(S, HID))
    hT = np.ascontiguousarray(h.T)
    cos2 = np.asarray(cos, np.float32).reshape(S, D)
    sin2 = np.asarray(sin, np.float32).reshape(S, D)
    cosT = np.ascontiguousarray(cos2.T)
    sgn = np.where(np.arange(D) < D // 2, -1.0, 1.0).astype(np.float32)
    sinT = np.ascontiguousarray((sin2 * sgn).T)

    mask2 = np.asarray(attention_mask, np.float32).reshape(S, KV)
    # The kernel hardcodes the causal block structure; verify it holds.
    expect = np.tril(np.ones((S, KV), np.float32))
    if not np.array_equal(mask2, expect):
        raise ValueError("kernel compiled for a causal (tril) attention_mask")
    bias4 = np.empty((4, 128, QCH), np.float32)
    for t in range(4):
        sub = mask2[0:QCH, t * 128 : (t + 1) * 128]  # [q, kv_local]
        bias4[t] = np.where(sub.T > 0.5, 0.0, NEG).astype(np.float32)

    scale = 1.0 / math.sqrt(D)
    Wq = np.asarray(Wq, np.float32)
    Wk = np.asarray(Wk, np.float32)
    Wv = np.asarray(Wv, np.float32)
    Wo = np.asarray(Wo, np.float32)

    in_maps = []
    for c in range(N_CORES):
        wq_c = np.ascontiguousarray((Wq[c * G * D : (c + 1) * G * D, :] * scale).T)
        wk_c = np.ascontiguousarray(Wk[c * D : (c + 1) * D, :].T)
        wv_c = np.ascontiguousarray(Wv[c * D : (c + 1) * D, :].T)
        wo_c = np.ascontiguousarray(Wo[:, c * G * D : (c + 1) * G * D].T)
        in_maps.append(
            dict(
                hT=hT,
                wqT=wq_c,
                wkT=wk_c,
                wvT=wv_c,
                woT=wo_c,
                cosT=cosT,
                sinT=sinT,
                bias4=bias4,
            )
        )
    return in_maps


def kernel(
    hidden_states,
    attention_mask,
    cos,
    sin,
    past_key,
    past_value,
    Wq,
    Wk,
    Wv,
    Wo,
    seq_positions,
    batch_position,
):
    global _compiled, LAST_RESULT
    assert int(np.asarray(seq_positions).reshape(-1)[0]) == 0
    assert int(np.asarray(batch_position)) == 0

    if _compiled is None:
        _compiled = _build()
    nc = _compiled

    in_maps = _prep_inputs(hidden_states, attention_mask, cos, sin, Wq, Wk, Wv, Wo)
    res = run_bass_kernel_spmd(nc, in_maps, list(range(N_CORES)), trace=TRACE)
    LAST_RESULT = res

    pieces = PIECES
    key_cache = np.empty((1, KVH, KV, D), np.float32)
    value_cache = np.empty((1, KVH, KV, D), np.float32)
    attn_out = np.empty((S, HID), np.float32)
    for c in range(N_CORES):
        r = res.results[c]
        key_cache[0, c] = r["kT_out"].T
        value_cache[0, c] = r["v_out"]
        for r0, n in pieces:
            sh = n // N_CORES
            attn_out[r0 + sh * c : r0 + sh * (c + 1)] = r["o_out"][
                r0 // N_CORES : r0 // N_CORES + sh
            ]
    return attn_out.reshape(1, S, HID), key_cache, value_cache


# revision 59
# speedup vs baseline: 1.1359x; 1.0069x over previous
"""Trainium2 Bass kernel for decoder-only GQA attention (tensor-parallel x8).

Problem (hardcoded): B=1, S=2048, HID=4096, H=32 q-heads, KVH=8 kv-heads,
D=128, KV_LEN=2048, seq_position=0, batch_position=0, causal mask.

Sharding: tensor-parallel over the 8 kv heads.  Core c owns kv head c and
q heads 4c..4c+3.  Wq/Wk/Wv sharded along their output (head) dim, Wo along
its input dim.  Each core computes a partial o_proj output [2048, 4096];
a per-chunk ReduceScatter sums the partials and leaves row-shard slices
that the host reassembles (the "gather" half of the hinted all-reduce is
done by the host-side unshard).

Device-side dataflow is entirely "transposed" to keep every matmul
transpose-free:
  hiddenT [hid, s] -> QT/KT [d, s] (RoPE applied in the transposed layout
  via a partition-rotation DMA and sign-folded sin), VT -> V via PE
  transpose, scoresT [kv, q] = K @ QT, softmax along the partition (kv)
  axis with the denominator computed by a ones-column matmul, ctxT [d, q]
  = V.T @ expST, o [q, hid] = ctxT.T @ WoT.

Matmuls run in float32r (fp32 with reduced mantissa, 4x the fp32 rate,
~1e-4 matmul error); everything else is fp32.
"""

import math

import numpy as np

import concourse.bacc as bacc
import concourse.mybir as mybir
import concourse.tile as tile
from concourse.bass_utils import run_bass_kernel_spmd
from concourse.masks import make_identity

S = 2048
HID = 4096
H = 32
KVH = 8
D = 128
G = H // KVH  # q heads per core
KV = 2048
N_CORES = 8
QCH = 512  # q-rows per chunk
NCH = S // QCH  # 4 chunks
HT = HID // 128  # 32 h-tiles
NEG = -1.0e9

F32 = mybir.dt.float32
F32R = mybir.dt.float32r

# Set by test.py to collect HW timing/profiles.
TRACE = False
LAST_RESULT = None

# ReduceScatter pieces (row ranges of attn_out), shared by device and host code.
# Smaller tail pieces shrink the exposed latency of the final collective.
PIECES = [(r0, 256) for r0 in range(0, S - 256, 256)] + [(S - 256, 128), (S - 128, 128)]

_compiled = None


def _build():
    nc = bacc.Bacc("TRN2", target_bir_lowering=False, num_devices=N_CORES)

    hT = nc.declare_dram_parameter("hT", [HID, S], F32R, isOutput=False)
    wqT = nc.declare_dram_parameter("wqT", [HID, G * D], F32R, isOutput=False)
    wkT = nc.declare_dram_parameter("wkT", [HID, D], F32R, isOutput=False)
    wvT = nc.declare_dram_parameter("wvT", [HID, D], F32R, isOutput=False)
    woT = nc.declare_dram_parameter("woT", [G * D, HID], F32R, isOutput=False)
    cosT = nc.declare_dram_parameter("cosT", [D, S], F32, isOutput=False)
    sinT = nc.declare_dram_parameter("sinT", [D, S], F32, isOutput=False)
    bias4 = nc.declare_dram_parameter("bias4", [4, 128, QCH], F32, isOutput=False)

    kT_out = nc.declare_dram_parameter("kT_out", [D, S], F32, isOutput=True)
    v_out = nc.declare_dram_parameter("v_out", [KV, D], F32, isOutput=True)
    o_out = nc.declare_dram_parameter("o_out", [S // N_CORES, HID], F32, isOutput=True)

    from contextlib import ExitStack

    with tile.TileContext(nc) as tc, ExitStack() as ctx_stack:
        _body(
            nc, tc, ctx_stack, hT, wqT, wkT, wvT, woT, cosT, sinT, bias4,
            kT_out, v_out, o_out,
        )
    nc.compile()
    return nc


def _body(nc, tc, ctx_stack, hT, wqT, wkT, wvT, woT, cosT, sinT, bias4, kT_out, v_out, o_out):
    ec = ctx_stack.enter_context
    persist = ec(tc.tile_pool(name="persist", bufs=1))
    wbig = ec(tc.tile_pool(name="wbig", bufs=1))
    stream = ec(tc.tile_pool(name="stream", bufs=4))
    tmp = ec(tc.tile_pool(name="tmp", bufs=4))
    es_pool = ec(tc.tile_pool(name="es", bufs=3))
    ctx_pool = ec(tc.tile_pool(name="ctx", bufs=4))
    osb_pool = ec(tc.tile_pool(name="osb", bufs=2))
    dram = ec(tc.tile_pool(name="dram", bufs=1, space="DRAM"))

    # --- persistent SBUF ---
    ident = persist.tile([128, 128], F32)
    make_identity(nc, ident[:])
    ones_f32 = persist.tile([128, 1], F32)
    nc.vector.memset(ones_f32[:], 1.0)
    ones_col = persist.tile([128, 1], F32R)
    nc.vector.tensor_copy(ones_col[:], ones_f32[:])
    onesr_f32 = persist.tile([1, 128], F32)
    nc.vector.memset(onesr_f32[:], 1.0)
    ones_row = persist.tile([1, 128], F32R)
    nc.vector.tensor_copy(ones_row[:], onesr_f32[:])

    wk_sb = persist.tile([128, HT * D], F32R)  # [p, (t d)]
    wv_sb = persist.tile([128, HT * D], F32R)
    bias_sb = persist.tile([128, 4 * QCH], F32)
    nc.gpsimd.dma_start(
        out=bias_sb[:].rearrange("p (t n) -> p t n", n=QCH),
        in_=bias4.rearrange("t p n -> p t n"),
    )

    # per-(head, chunk) / per-chunk tiles: Tile tracks dependencies at tile
    # granularity, so one big tile would make chunk-0 attention wait for the
    # chunk-3 RoPE epilogue.
    qT_c = [
        [persist.tile([128, QCH], F32R, name=f"qT_{h}_{j}") for j in range(NCH)]
        for h in range(G)
    ]
    kT_c = [persist.tile([128, QCH], F32R, name=f"kT_{j}") for j in range(NCH)]
    v_c = [persist.tile([128, QCH], F32R, name=f"v_{j}") for j in range(NCH)]

    # wq is only needed during the projection phase, wo afterwards; they share
    # one 8MB slot.  Per-h-tile slice DMAs, interleaved wq/wk/wv in t order,
    # so the t=0 matmuls start as soon as the first three slices land.
    wq_sb = wbig.tile([128, HT * G * D], F32R, tag="w")  # [p, (t m)]
    for t in range(HT):
        nc.gpsimd.dma_start(
            out=wq_sb[:, t * G * D : (t + 1) * G * D],
            in_=wqT[t * 128 : (t + 1) * 128, :],
        )
        nc.gpsimd.dma_start(
            out=wk_sb[:, t * D : (t + 1) * D], in_=wkT[t * 128 : (t + 1) * 128, :]
        )
        nc.gpsimd.dma_start(
            out=wv_sb[:, t * D : (t + 1) * D], in_=wvT[t * 128 : (t + 1) * 128, :]
        )

    # RS pieces: big early pieces overlap compute; small tail pieces cut the
    # exposed latency of the final collective.  One DRAM tile per piece —
    # a single big o_part tile would give later stores a false whole-tile
    # WAR dependency on each in-flight ReduceScatter.
    pieces = PIECES
    o_parts = [
        dram.tile([n, HID], F32, tag=f"opart{i}", name=f"o_part{i}")
        for i, (r0, n) in enumerate(pieces)
    ]
    o_shards = [
        dram.tile([n // N_CORES, HID], F32, tag=f"osh{i}", name=f"o_shard{i}")
        for i, (r0, n) in enumerate(pieces)
    ]

    def piece_of_row(row):
        for i, (r0, n) in enumerate(pieces):
            if r0 <= row < r0 + n:
                return i, r0
        raise AssertionError(row)

    # ---------------- Phase A: QKV projection + RoPE + V transpose ----------
    with tc.tile_pool(name="psA", bufs=1, space="PSUM") as psA:
        for j in range(NCH):
            q0 = QCH * j
            cos_sb = stream.tile([128, QCH], F32, tag="cs", bufs=2)
            sin_sb = stream.tile([128, QCH], F32, tag="cs", bufs=2)
            nc.gpsimd.dma_start(out=cos_sb[:], in_=cosT[:, q0 : q0 + QCH])
            nc.gpsimd.dma_start(out=sin_sb[:], in_=sinT[:, q0 : q0 + QCH])

            ps = [
                psA.tile([128, QCH], F32, tag="proj", name=f"proj{j}_{m}", bufs=6)
                for m in range(6)
            ]
            for t in range(HT):
                ht = stream.tile([128, QCH], F32R, tag="ht", bufs=6)
                nc.sync.dma_start(
                    out=ht[:], in_=hT[t * 128 : (t + 1) * 128, q0 : q0 + QCH]
                )
                fl = dict(start=(t == 0), stop=(t == HT - 1), skip_group_check=True)
                for m in range(G):
                    nc.tensor.matmul(
                        ps[m][:],
                        wq_sb[:, t * G * D + m * 128 : t * G * D + (m + 1) * 128],
                        ht[:],
                        **fl,
                    )
                nc.tensor.matmul(
                    ps[4][:], wk_sb[:, t * 128 : (t + 1) * 128], ht[:], **fl
                )
                nc.tensor.matmul(
                    ps[5][:], wv_sb[:, t * 128 : (t + 1) * 128], ht[:], **fl
                )

            # V first (its ACT copy unblocks the PE transposes that sit next
            # in the PE instruction stream), then RoPE for the 4 q heads + k.
            vt = tmp.tile([128, QCH], F32, tag="scratch")
            nc.scalar.copy(vt[:], ps[5][:])
            for b in range(QCH // 128):
                kvi = 4 * j + b
                pst = psA.tile([128, 128], F32, tag="tr", bufs=2)
                nc.tensor.transpose(pst[:], vt[:, b * 128 : (b + 1) * 128], ident[:])
                nc.vector.tensor_copy(v_c[j][:, b * 128 : (b + 1) * 128], pst[:])
                nc.scalar.dma_start(
                    out=v_out[kvi * 128 : (kvi + 1) * 128, :],
                    in_=v_c[j][:, b * 128 : (b + 1) * 128].bitcast(F32),
                )

            # RoPE for the 4 q heads and k: out = ps*cos + rot(ps)*sin_eff
            for m in range(5):
                src = ps[m] if m < G else ps[4]
                if m < G:
                    dst = qT_c[m][j][:]
                else:
                    dst = kT_c[j][:]
                qraw = tmp.tile([128, QCH], F32, tag="scratch")
                nc.scalar.copy(qraw[:], src[:])
                perm = tmp.tile([128, QCH], F32, tag="scratch")
                nc.scalar.dma_start(out=perm[0:64, :], in_=qraw[64:128, :])
                nc.scalar.dma_start(out=perm[64:128, :], in_=qraw[0:64, :])
                # in-place: qraw *= cos, perm *= sin_eff, dst = qraw + perm
                nc.vector.tensor_mul(qraw[:], qraw[:], cos_sb[:])
                nc.vector.tensor_mul(perm[:], perm[:], sin_sb[:])
                nc.vector.tensor_add(dst, qraw[:], perm[:])
            nc.scalar.dma_start(
                out=kT_out[:, q0 : q0 + QCH], in_=kT_c[j][:].bitcast(F32)
            )

    # wo replaces wq in the shared slot; per-dh slices so o_proj dh=0 starts early
    wo_sb = wbig.tile([128, G * HID], F32R, tag="w")  # [p, (dh n)]
    for dh in range(G):
        nc.gpsimd.dma_start(
            out=wo_sb[:, dh * HID : (dh + 1) * HID],
            in_=woT[dh * 128 : (dh + 1) * 128, :],
        )

    # ---------------- Phase B: attention + o_proj + ReduceScatter -----------
    with tc.tile_pool(name="psB", bufs=1, space="PSUM") as psB:
        for j in range(NCH):
            q0 = QCH * j
            nkv = (q0 + QCH) // 128  # causal: kv tiles 0..nkv-1
            ctxs = [None] * G
            pending = None  # (ctx_ps, den_ps, h) awaiting normalize

            def normalize(ctx_ps, den_ps, h):
                # deferred by one head so the bc matmul (which waits on the
                # DVE reciprocal) doesn't stall the in-order PE stream
                inv = tmp.tile([1, QCH], F32R, tag="inv", bufs=2)
                with nc.allow_low_precision(reason="f32r softmax denom"):
                    nc.vector.reciprocal(inv[:], den_ps[:])
                bc_ps = psB.tile([128, QCH], F32, tag="s", bufs=4)
                nc.tensor.matmul(bc_ps[:], ones_row[:], inv[:], start=True, stop=True)
                ctxc = tmp.tile([128, QCH], F32, tag="scratch")
                nc.scalar.copy(ctxc[:], ctx_ps[:])
                ctx_sb = ctx_pool.tile([128, QCH], F32R, tag="ctx_sb")
                nc.vector.tensor_mul(ctx_sb[:], ctxc[:], bc_ps[:])
                ctxs[h] = ctx_sb

            for h in range(G):
                ctx_ps = psB.tile([128, QCH], F32, tag="ctx", bufs=2)
                den_ps = psB.tile([1, QCH], F32, tag="den", bufs=2)
                for kv in range(nkv):
                    jc, b = kv // 4, kv % 4
                    s_ps = psB.tile([128, QCH], F32, tag="s", bufs=4)
                    nc.tensor.matmul(
                        s_ps[:],
                        kT_c[jc][:, b * 128 : (b + 1) * 128],
                        qT_c[h][j][:],
                        start=True,
                        stop=True,
                    )
                    db = kv - (nkv - 4)
                    if db >= 0:  # diagonal band: apply mask bias
                        nc.vector.tensor_add(
                            s_ps[:], s_ps[:], bias_sb[:, db * QCH : (db + 1) * QCH]
                        )
                    es = es_pool.tile([128, QCH], F32R, tag="es")
                    nc.scalar.activation(
                        es[:], s_ps[:], mybir.ActivationFunctionType.Exp
                    )
                    flk = dict(
                        start=(kv == 0), stop=(kv == nkv - 1), skip_group_check=True
                    )
                    nc.tensor.matmul(
                        ctx_ps[:], v_c[jc][:, b * 128 : (b + 1) * 128], es[:], **flk
                    )
                    nc.tensor.matmul(den_ps[:], ones_col[:], es[:], **flk)
                if pending is not None:
                    normalize(*pending)
                pending = (ctx_ps, den_ps, h)
            normalize(*pending)

            for qt in range(QCH // 128):
                for nh in range(HID // QCH):
                    o_ps = psB.tile([128, QCH], F32, tag="s", bufs=4)
                    for dh in range(G):
                        nc.tensor.matmul(
                            o_ps[:],
                            ctxs[dh][:, qt * 128 : (qt + 1) * 128],
                            wo_sb[:, dh * HID + nh * QCH : dh * HID + (nh + 1) * QCH],
                            start=(dh == 0),
                            stop=(dh == G - 1),
                            skip_group_check=True,
                        )
                    o_sb = osb_pool.tile([128, QCH], F32, tag="osb")
                    nc.vector.tensor_copy(o_sb[:], o_ps[:])
                    row = q0 + qt * 128
                    pi, pr0 = piece_of_row(row)
                    nc.scalar.dma_start(
                        out=o_parts[pi][
                            row - pr0 : row - pr0 + 128, nh * QCH : (nh + 1) * QCH
                        ],
                        in_=o_sb[:],
                    )
                # fire the RS for any piece whose rows are now fully written
                row_end = q0 + (qt + 1) * 128
                for i, (r0, n) in enumerate(pieces):
                    if r0 + n == row_end:
                        nc.gpsimd.collective_compute(
                            "ReduceScatter",
                            mybir.AluOpType.add,
                            replica_groups=[list(range(N_CORES))],
                            ins=[o_parts[i][:]],
                            outs=[o_shards[i][:]],
                        )
                        # sync queue: a store waiting on the RS would
                        # head-of-line-block the scalar queue's o_part stores
                        nc.sync.dma_start(
                            out=o_out[r0 // N_CORES : (r0 + n) // N_CORES, :],
                            in_=o_shards[i][:],
                        )


def _prep_inputs(hidden_states, attention_mask, cos, sin, Wq, Wk, Wv, Wo):
    h = np.ascontiguousarray(np.asarray(hidden_states, np.float32).reshape(S, HID))
    hT = np.ascontiguousarray(h.T)
    cos2 = np.asarray(cos, np.float32).reshape(S, D)
    sin2 = np.asarray(sin, np.float32).reshape(S, D)
    cosT = np.ascontiguousarray(cos2.T)
    sgn = np.where(np.arange(D) < D // 2, -1.0, 1.0).astype(np.float32)
    sinT = np.ascontiguousarray((sin2 * sgn).T)

    mask2 = np.asarray(attention_mask, np.float32).reshape(S, KV)
    # The kernel hardcodes the causal block structure; verify it holds.
    expect = np.tril(np.ones((S, KV), np.float32))
    if not np.array_equal(mask2, expect):
        raise ValueError("kernel compiled for a causal (tril) attention_mask")
    bias4 = np.empty((4, 128, QCH), np.float32)
    for t in range(4):
        sub = mask2[0:QCH, t * 128 : (t + 1) * 128]  # [q, kv_local]
        bias4[t] = np.where(sub.T > 0.5, 0.0, NEG).astype(np.float32)

    scale = 1.0 / math.sqrt(D)
    Wq = np.asarray(Wq, np.float32)
    Wk = np.asarray(Wk, np.float32)
    Wv = np.asarray(Wv, np.float32)
    Wo = np.asarray(Wo, np.float32)

    in_maps = []
    for c in range(N_CORES):
        wq_c = np.ascontiguousarray((Wq[c * G * D : (c + 1) * G * D, :] * scale).T)
        wk_c = np.ascontiguousarray(Wk[c * D : (c + 1) * D, :].T)
        wv_c = np.ascontiguousarray(Wv[c * D : (c + 1) * D, :].T)
        wo_c = np.ascontiguousarray(Wo[:, c * G * D : (c + 1) * G * D].T)
        in_maps.append(
            dict(
                hT=hT,
                wqT=wq_c,
                wkT=wk_c,
                wvT=wv_c,
                woT=wo_c,
                cosT=cosT,
                sinT=sinT,
                bias4=bias4,
            )
        )
    return in_maps


def kernel(
    hidden_states,
    attention_mask,
    cos,
    sin,
    past_key,
    past_value,
    Wq,
    Wk,
    Wv,
    Wo,
    seq_positions,
    batch_position,
):
    global _compiled, LAST_RESULT
    assert int(np.asarray(seq_positions).reshape(-1)[0]) == 0
    assert int(np.asarray(batch_position)) == 0

    if _compiled is None:
        _compiled = _build()
    nc = _compiled

    in_maps = _prep_inputs(hidden_states, attention_mask, cos, sin, Wq, Wk, Wv, Wo)
    res = run_bass_kernel_spmd(nc, in_maps, list(range(N_CORES)), trace=TRACE)
    LAST_RESULT = res

    pieces = PIECES
    key_cache = np.empty((1, KVH, KV, D), np.float32)
    value_cache = np.empty((1, KVH, KV, D), np.float32)
    attn_out = np.empty((S, HID), np.float32)
    for c in range(N_CORES):
        r = res.results[c]
        key_cache[0, c] = r["kT_out"].T
        value_cache[0, c] = r["v_out"]
        for r0, n in pieces:
            sh = n // N_CORES
            attn_out[r0 + sh * c : r0 + sh * (c + 1)] = r["o_out"][
                r0 // N_CORES : r0 // N_CORES + sh
            ]
    return attn_out.reshape(1, S, HID), key_cache, value_cache


# revision 67
# speedup vs baseline: 1.2872x; 1.1332x over previous
"""Trainium2 Bass kernel for decoder-only GQA attention (tensor-parallel x8).

Problem (hardcoded): B=1, S=2048, HID=4096, H=32 q-heads, KVH=8 kv-heads,
D=128, KV_LEN=2048, seq_position=0, batch_position=0, causal mask.

Sharding: tensor-parallel over the 8 kv heads.  Core c owns kv head c and
q heads 4c..4c+3.  Wq/Wk/Wv sharded along their output (head) dim, Wo along
its input dim.  Each core computes a partial o_proj output [2048, 4096];
a per-chunk ReduceScatter sums the partials and leaves row-shard slices
that the host reassembles (the "gather" half of the hinted all-reduce is
done by the host-side unshard).

Device-side dataflow is entirely "transposed" to keep every matmul
transpose-free:
  hiddenT [hid, s] -> QT/KT [d, s] (RoPE applied in the transposed layout
  via a partition-rotation DMA and sign-folded sin), VT -> V via PE
  transpose, scoresT [kv, q] = K @ QT, softmax along the partition (kv)
  axis with the denominator computed by a ones-column matmul, ctxT [d, q]
  = V.T @ expST, o [q, hid] = ctxT.T @ WoT.

Matmuls run in float32r (fp32 with reduced mantissa, 4x the fp32 rate,
~1e-4 matmul error); everything else is fp32.
"""

import math

import numpy as np

import concourse.bacc as bacc
import concourse.mybir as mybir
import concourse.tile as tile
from concourse.bass_utils import run_bass_kernel_spmd
from concourse.masks import make_identity

S = 2048
HID = 4096
H = 32
KVH = 8
D = 128
G = H // KVH  # q heads per core
KV = 2048
N_CORES = 8
QCH = 512  # q-rows per chunk
NCH = S // QCH  # 4 chunks
HT = HID // 128  # 32 h-tiles
NEG = -1.0e9

F32 = mybir.dt.float32
F32R = mybir.dt.float32r
BF16 = mybir.dt.bfloat16

# Set by test.py to collect HW timing/profiles.
TRACE = False
LAST_RESULT = None

# ReduceScatter pieces (row ranges of attn_out), shared by device and host code.
# Smaller tail pieces shrink the exposed latency of the final collective.
PIECES = [
    (0, 512), (512, 512), (1024, 512), (1536, 256), (1792, 128), (1920, 128),
]
# bf16 partial-sums for the o_proj ReduceScatter: halves collective traffic
# (the mesh bursts starve app DMA) at ~2e-3 attn_out error.
RS_BF16 = True
ODT = BF16 if RS_BF16 else F32

_compiled = None


def _build():
    nc = bacc.Bacc("TRN2", target_bir_lowering=False, num_devices=N_CORES)

    hT = nc.declare_dram_parameter("hT", [HID, S], F32R, isOutput=False)
    wqT = nc.declare_dram_parameter("wqT", [HID, G * D], F32R, isOutput=False)
    wkT = nc.declare_dram_parameter("wkT", [HID, D], F32R, isOutput=False)
    wvT = nc.declare_dram_parameter("wvT", [HID, D], F32R, isOutput=False)
    woT = nc.declare_dram_parameter("woT", [G * D, HID], F32R, isOutput=False)
    cosT = nc.declare_dram_parameter("cosT", [D, S], F32, isOutput=False)
    sinT = nc.declare_dram_parameter("sinT", [D, S], F32, isOutput=False)
    bias4 = nc.declare_dram_parameter("bias4", [4, 128, QCH], F32, isOutput=False)

    kT_out = nc.declare_dram_parameter("kT_out", [D, S], F32, isOutput=True)
    v_out = nc.declare_dram_parameter("v_out", [KV, D], F32, isOutput=True)
    o_out = nc.declare_dram_parameter("o_out", [S // N_CORES, HID], ODT, isOutput=True)

    from contextlib import ExitStack

    with tile.TileContext(nc) as tc, ExitStack() as ctx_stack:
        _body(
            nc, tc, ctx_stack, hT, wqT, wkT, wvT, woT, cosT, sinT, bias4,
            kT_out, v_out, o_out,
        )
    nc.compile()
    return nc


def _body(nc, tc, ctx_stack, hT, wqT, wkT, wvT, woT, cosT, sinT, bias4, kT_out, v_out, o_out):
    ec = ctx_stack.enter_context
    persist = ec(tc.tile_pool(name="persist", bufs=1))
    wbig = ec(tc.tile_pool(name="wbig", bufs=1))
    stream = ec(tc.tile_pool(name="stream", bufs=4))
    tmp = ec(tc.tile_pool(name="tmp", bufs=4))
    es_pool = ec(tc.tile_pool(name="es", bufs=3))
    ctx_pool = ec(tc.tile_pool(name="ctx", bufs=4))
    osb_pool = ec(tc.tile_pool(name="osb", bufs=2))
    dram = ec(tc.tile_pool(name="dram", bufs=1, space="DRAM"))

    # --- persistent SBUF ---
    ident = persist.tile([128, 128], F32)
    make_identity(nc, ident[:])
    ones_f32 = persist.tile([128, 1], F32)
    nc.vector.memset(ones_f32[:], 1.0)
    ones_col = persist.tile([128, 1], F32R)
    nc.vector.tensor_copy(ones_col[:], ones_f32[:])
    onesr_f32 = persist.tile([1, 128], F32)
    nc.vector.memset(onesr_f32[:], 1.0)
    ones_row = persist.tile([1, 128], F32R)
    nc.vector.tensor_copy(ones_row[:], onesr_f32[:])

    wk_sb = persist.tile([128, HT * D], F32R)  # [p, (t d)]
    wv_sb = persist.tile([128, HT * D], F32R)
    bias_sb = persist.tile([128, 4 * QCH], F32)
    nc.gpsimd.dma_start(
        out=bias_sb[:].rearrange("p (t n) -> p t n", n=QCH),
        in_=bias4.rearrange("t p n -> p t n"),
    )

    # per-(head, chunk) / per-chunk tiles: Tile tracks dependencies at tile
    # granularity, so one big tile would make chunk-0 attention wait for the
    # chunk-3 RoPE epilogue.
    qT_c = [
        [persist.tile([128, QCH], F32R, name=f"qT_{h}_{j}") for j in range(NCH)]
        for h in range(G)
    ]
    kT_c = [persist.tile([128, QCH], F32R, name=f"kT_{j}") for j in range(NCH)]
    v_c = [persist.tile([128, QCH], F32R, name=f"v_{j}") for j in range(NCH)]

    # wq is only needed during the projection phase, wo afterwards; they share
    # one 8MB slot.  Per-h-tile slice DMAs, interleaved wq/wk/wv in t order,
    # so the t=0 matmuls start as soon as the first three slices land.
    wq_sb = wbig.tile([128, HT * G * D], F32R, tag="w")  # [p, (t m)]
    for t in range(HT):
        nc.gpsimd.dma_start(
            out=wq_sb[:, t * G * D : (t + 1) * G * D],
            in_=wqT[t * 128 : (t + 1) * 128, :],
        )
        nc.gpsimd.dma_start(
            out=wk_sb[:, t * D : (t + 1) * D], in_=wkT[t * 128 : (t + 1) * 128, :]
        )
        nc.gpsimd.dma_start(
            out=wv_sb[:, t * D : (t + 1) * D], in_=wvT[t * 128 : (t + 1) * 128, :]
        )

    # RS pieces: big early pieces overlap compute; small tail pieces cut the
    # exposed latency of the final collective.  One DRAM tile per piece —
    # a single big o_part tile would give later stores a false whole-tile
    # WAR dependency on each in-flight ReduceScatter.
    pieces = PIECES
    o_parts = [
        dram.tile([n, HID], ODT, tag=f"opart{i}", name=f"o_part{i}")
        for i, (r0, n) in enumerate(pieces)
    ]
    o_shards = [
        dram.tile([n // N_CORES, HID], ODT, tag=f"osh{i}", name=f"o_shard{i}")
        for i, (r0, n) in enumerate(pieces)
    ]

    def piece_of_row(row):
        for i, (r0, n) in enumerate(pieces):
            if r0 <= row < r0 + n:
                return i, r0
        raise AssertionError(row)

    # ---------------- Phase A: QKV projection + RoPE + V transpose ----------
    with tc.tile_pool(name="psA", bufs=1, space="PSUM") as psA:
        for j in range(NCH):
            q0 = QCH * j
            cos_sb = stream.tile([128, QCH], F32, tag="cs", bufs=2)
            sin_sb = stream.tile([128, QCH], F32, tag="cs", bufs=2)
            nc.gpsimd.dma_start(out=cos_sb[:], in_=cosT[:, q0 : q0 + QCH])
            nc.gpsimd.dma_start(out=sin_sb[:], in_=sinT[:, q0 : q0 + QCH])

            ps = [
                psA.tile([128, QCH], F32, tag="proj", name=f"proj{j}_{m}", bufs=6)
                for m in range(6)
            ]
            for t in range(HT):
                ht = stream.tile([128, QCH], F32R, tag="ht", bufs=6)
                nc.sync.dma_start(
                    out=ht[:], in_=hT[t * 128 : (t + 1) * 128, q0 : q0 + QCH]
                )
                fl = dict(start=(t == 0), stop=(t == HT - 1), skip_group_check=True)
                for m in range(G):
                    nc.tensor.matmul(
                        ps[m][:],
                        wq_sb[:, t * G * D + m * 128 : t * G * D + (m + 1) * 128],
                        ht[:],
                        **fl,
                    )
                nc.tensor.matmul(
                    ps[4][:], wk_sb[:, t * 128 : (t + 1) * 128], ht[:], **fl
                )
                nc.tensor.matmul(
                    ps[5][:], wv_sb[:, t * 128 : (t + 1) * 128], ht[:], **fl
                )

            # V first (its ACT copy unblocks the PE transposes that sit next
            # in the PE instruction stream), then RoPE for the 4 q heads + k.
            vt = tmp.tile([128, QCH], F32, tag="scratch")
            nc.scalar.copy(vt[:], ps[5][:])
            for b in range(QCH // 128):
                kvi = 4 * j + b
                pst = psA.tile([128, 128], F32, tag="tr", bufs=2)
                nc.tensor.transpose(pst[:], vt[:, b * 128 : (b + 1) * 128], ident[:])
                nc.vector.tensor_copy(v_c[j][:, b * 128 : (b + 1) * 128], pst[:])
                nc.scalar.dma_start(
                    out=v_out[kvi * 128 : (kvi + 1) * 128, :],
                    in_=v_c[j][:, b * 128 : (b + 1) * 128].bitcast(F32),
                )

            # RoPE for the 4 q heads and k: out = ps*cos + rot(ps)*sin_eff
            for m in range(5):
                src = ps[m] if m < G else ps[4]
                if m < G:
                    dst = qT_c[m][j][:]
                else:
                    dst = kT_c[j][:]
                # alternate engines/queues so the five per-head chains overlap
                qraw = tmp.tile([128, QCH], F32, tag="scratch")
                if m % 2 == 0:
                    nc.scalar.copy(qraw[:], src[:])
                else:
                    nc.vector.tensor_copy(qraw[:], src[:])
                perm = tmp.tile([128, QCH], F32, tag="scratch")
                dq = nc.scalar if m % 2 == 0 else nc.sync
                dq.dma_start(out=perm[0:64, :], in_=qraw[64:128, :])
                dq.dma_start(out=perm[64:128, :], in_=qraw[0:64, :])
                # in-place: qraw *= cos, perm *= sin_eff, dst = qraw + perm
                nc.vector.tensor_mul(qraw[:], qraw[:], cos_sb[:])
                nc.vector.tensor_mul(perm[:], perm[:], sin_sb[:])
                nc.vector.tensor_add(dst, qraw[:], perm[:])
            nc.scalar.dma_start(
                out=kT_out[:, q0 : q0 + QCH], in_=kT_c[j][:].bitcast(F32)
            )

    # wo replaces wq in the shared slot; per-dh slices so o_proj dh=0 starts early
    wo_sb = wbig.tile([128, G * HID], F32R, tag="w")  # [p, (dh n)]
    for dh in range(G):
        nc.gpsimd.dma_start(
            out=wo_sb[:, dh * HID : (dh + 1) * HID],
            in_=woT[dh * 128 : (dh + 1) * 128, :],
        )

    # ---------------- Phase B: attention + o_proj + ReduceScatter -----------
    with tc.tile_pool(name="psB", bufs=1, space="PSUM") as psB:
        for j in range(NCH):
            q0 = QCH * j
            nkv = (q0 + QCH) // 128  # causal: kv tiles 0..nkv-1
            ctxs = [None] * G
            pending = None  # (ctx_ps, den_ps, h) awaiting normalize

            def normalize(ctx_ps, den_ps, h):
                # deferred by one head so the bc matmul (which waits on the
                # DVE reciprocal) doesn't stall the in-order PE stream
                inv = tmp.tile([1, QCH], F32R, tag="inv", bufs=2)
                with nc.allow_low_precision(reason="f32r softmax denom"):
                    nc.vector.reciprocal(inv[:], den_ps[:])
                bc_ps = psB.tile([128, QCH], F32, tag="s", bufs=4)
                nc.tensor.matmul(bc_ps[:], ones_row[:], inv[:], start=True, stop=True)
                ctxc = tmp.tile([128, QCH], F32, tag="scratch")
                nc.scalar.copy(ctxc[:], ctx_ps[:])
                ctx_sb = ctx_pool.tile([128, QCH], F32R, tag="ctx_sb")
                nc.vector.tensor_mul(ctx_sb[:], ctxc[:], bc_ps[:])
                ctxs[h] = ctx_sb

            for h in range(G):
                ctx_ps = psB.tile([128, QCH], F32, tag="ctx", bufs=2)
                den_ps = psB.tile([1, QCH], F32, tag="den", bufs=2)
                for kv in range(nkv):
                    jc, b = kv // 4, kv % 4
                    s_ps = psB.tile([128, QCH], F32, tag="s", bufs=4)
                    nc.tensor.matmul(
                        s_ps[:],
                        kT_c[jc][:, b * 128 : (b + 1) * 128],
                        qT_c[h][j][:],
                        start=True,
                        stop=True,
                    )
                    db = kv - (nkv - 4)
                    if db >= 0:  # diagonal band: apply mask bias
                        nc.vector.tensor_add(
                            s_ps[:], s_ps[:], bias_sb[:, db * QCH : (db + 1) * QCH]
                        )
                    es = es_pool.tile([128, QCH], F32R, tag="es")
                    nc.scalar.activation(
                        es[:], s_ps[:], mybir.ActivationFunctionType.Exp
                    )
                    flk = dict(
                        start=(kv == 0), stop=(kv == nkv - 1), skip_group_check=True
                    )
                    nc.tensor.matmul(
                        ctx_ps[:], v_c[jc][:, b * 128 : (b + 1) * 128], es[:], **flk
                    )
                    nc.tensor.matmul(den_ps[:], ones_col[:], es[:], **flk)
                if pending is not None:
                    normalize(*pending)
                pending = (ctx_ps, den_ps, h)
            normalize(*pending)

            for qt in range(QCH // 128):
                for nh in range(HID // QCH):
                    o_ps = psB.tile([128, QCH], F32, tag="s", bufs=4)
                    for dh in range(G):
                        nc.tensor.matmul(
                            o_ps[:],
                            ctxs[dh][:, qt * 128 : (qt + 1) * 128],
                            wo_sb[:, dh * HID + nh * QCH : dh * HID + (nh + 1) * QCH],
                            start=(dh == 0),
                            stop=(dh == G - 1),
                            skip_group_check=True,
                        )
                    o_sb = osb_pool.tile([128, QCH], ODT, tag="osb")
                    nc.vector.tensor_copy(o_sb[:], o_ps[:])
                    row = q0 + qt * 128
                    pi, pr0 = piece_of_row(row)
                    nc.scalar.dma_start(
                        out=o_parts[pi][
                            row - pr0 : row - pr0 + 128, nh * QCH : (nh + 1) * QCH
                        ],
                        in_=o_sb[:],
                    )
                # fire the RS for any piece whose rows are now fully written
                row_end = q0 + (qt + 1) * 128
                for i, (r0, n) in enumerate(pieces):
                    if r0 + n == row_end:
                        nc.gpsimd.collective_compute(
                            "ReduceScatter",
                            mybir.AluOpType.add,
                            replica_groups=[list(range(N_CORES))],
                            ins=[o_parts[i][:]],
                            outs=[o_shards[i][:]],
                        )
                        # sync queue: a store waiting on the RS would
                        # head-of-line-block the scalar queue's o_part stores
                        nc.sync.dma_start(
                            out=o_out[r0 // N_CORES : (r0 + n) // N_CORES, :],
                            in_=o_shards[i][:],
                        )


def _prep_inputs(hidden_states, attention_mask, cos, sin, Wq, Wk, Wv, Wo):
    h = np.ascontiguousarray(np.asarray(hidden_states, np.float32).reshape(S, HID))
    hT = np.ascontiguousarray(h.T)
    cos2 = np.asarray(cos, np.float32).reshape(S, D)
    sin2 = np.asarray(sin, np.float32).reshape(S, D)
    cosT = np.ascontiguousarray(cos2.T)
    sgn = np.where(np.arange(D) < D // 2, -1.0, 1.0).astype(np.float32)
    sinT = np.ascontiguousarray((sin2 * sgn).T)

    mask2 = np.asarray(attention_mask, np.float32).reshape(S, KV)
    # The kernel hardcodes the causal block structure; verify it holds.
    expect = np.tril(np.ones((S, KV), np.float32))
    if not np.array_equal(mask2, expect):
        raise ValueError("kernel compiled for a causal (tril) attention_mask")
    bias4 = np.empty((4, 128, QCH), np.float32)
    for t in range(4):
        sub = mask2[0:QCH, t * 128 : (t + 1) * 128]  # [q, kv_local]
        bias4[t] = np.where(sub.T > 0.5, 0.0, NEG).astype(np.float32)

    scale = 1.0 / math.sqrt(D)
    Wq = np.asarray(Wq, np.float32)
    Wk = np.asarray(Wk, np.float32)
    Wv = np.asarray(Wv, np.float32)
    Wo = np.asarray(Wo, np.float32)

    in_maps = []
    for c in range(N_CORES):
        wq_c = np.ascontiguousarray((Wq[c * G * D : (c + 1) * G * D, :] * scale).T)
        wk_c = np.ascontiguousarray(Wk[c * D : (c + 1) * D, :].T)
        wv_c = np.ascontiguousarray(Wv[c * D : (c + 1) * D, :].T)
        wo_c = np.ascontiguousarray(Wo[:, c * G * D : (c + 1) * G * D].T)
        in_maps.append(
            dict(
                hT=hT,
                wqT=wq_c,
                wkT=wk_c,
                wvT=wv_c,
                woT=wo_c,
                cosT=cosT,
                sinT=sinT,
                bias4=bias4,
            )
        )
    return in_maps


def kernel(
    hidden_states,
    attention_mask,
    cos,
    sin,
    past_key,
    past_value,
    Wq,
    Wk,
    Wv,
    Wo,
    seq_positions,
    batch_position,
):
    global _compiled, LAST_RESULT
    assert int(np.asarray(seq_positions).reshape(-1)[0]) == 0
    assert int(np.asarray(batch_position)) == 0

    if _compiled is None:
        _compiled = _build()
    nc = _compiled

    in_maps = _prep_inputs(hidden_states, attention_mask, cos, sin, Wq, Wk, Wv, Wo)
    res = run_bass_kernel_spmd(nc, in_maps, list(range(N_CORES)), trace=TRACE)
    LAST_RESULT = res

    pieces = PIECES
    key_cache = np.empty((1, KVH, KV, D), np.float32)
    value_cache = np.empty((1, KVH, KV, D), np.float32)
    attn_out = np.empty((S, HID), np.float32)
    for c in range(N_CORES):
        r = res.results[c]
        key_cache[0, c] = r["kT_out"].T
        value_cache[0, c] = r["v_out"]
        o_np = np.asarray(r["o_out"], np.float32)
        for r0, n in pieces:
            sh = n // N_CORES
            attn_out[r0 + sh * c : r0 + sh * (c + 1)] = o_np[
                r0 // N_CORES : r0 // N_CORES + sh
            ]
    return attn_out.reshape(1, S, HID), key_cache, value_cache


# revision 74
# speedup vs baseline: 1.3631x; 1.0590x over previous
"""Trainium2 Bass kernel for decoder-only GQA attention (tensor-parallel x8).

Problem (hardcoded): B=1, S=2048, HID=4096, H=32 q-heads, KVH=8 kv-heads,
D=128, KV_LEN=2048, seq_position=0, batch_position=0, causal mask.

Sharding: tensor-parallel over the 8 kv heads.  Core c owns kv head c and
q heads 4c..4c+3.  Wq/Wk/Wv sharded along their output (head) dim, Wo along
its input dim.  Each core computes a partial o_proj output [2048, 4096];
a per-chunk ReduceScatter sums the partials and leaves row-shard slices
that the host reassembles (the "gather" half of the hinted all-reduce is
done by the host-side unshard).

Device-side dataflow is entirely "transposed" to keep every matmul
transpose-free:
  hiddenT [hid, s] -> QT/KT [d, s] (RoPE applied in the transposed layout
  via a partition-rotation DMA and sign-folded sin), VT -> V via PE
  transpose, scoresT [kv, q] = K @ QT, softmax along the partition (kv)
  axis with the denominator computed by a ones-column matmul, ctxT [d, q]
  = V.T @ expST, o [q, hid] = ctxT.T @ WoT.

Matmuls run in float32r (fp32 with reduced mantissa, 4x the fp32 rate,
~1e-4 matmul error); everything else is fp32.
"""

import math

import numpy as np

import concourse.bacc as bacc
import concourse.mybir as mybir
import concourse.tile as tile
from concourse.bass_utils import run_bass_kernel_spmd
from concourse.masks import make_identity

S = 2048
HID = 4096
H = 32
KVH = 8
D = 128
G = H // KVH  # q heads per core
KV = 2048
N_CORES = 8
QCH = 512  # q-rows per chunk
NCH = S // QCH  # 4 chunks
HT = HID // 128  # 32 h-tiles
NEG = -1.0e9

F32 = mybir.dt.float32
F32R = mybir.dt.float32r
BF16 = mybir.dt.bfloat16

# Set by test.py to collect HW timing/profiles.
TRACE = False
LAST_RESULT = None

# ReduceScatter pieces (row ranges of attn_out), shared by device and host code.
# Smaller tail pieces shrink the exposed latency of the final collective.
PIECES = [
    (0, 512), (512, 512), (1024, 512), (1536, 256), (1792, 128), (1920, 128),
]
# bf16 partial-sums for the o_proj ReduceScatter: halves collective traffic
# (the mesh bursts starve app DMA) at ~2e-3 attn_out error.
RS_BF16 = True
ODT = BF16 if RS_BF16 else F32

_compiled = None


def _build():
    nc = bacc.Bacc("TRN2", target_bir_lowering=False, num_devices=N_CORES)

    hT = nc.declare_dram_parameter("hT", [HID, S], F32R, isOutput=False)
    wqT = nc.declare_dram_parameter("wqT", [HID, G * D], F32R, isOutput=False)
    wkT = nc.declare_dram_parameter("wkT", [HID, D], F32R, isOutput=False)
    wvT = nc.declare_dram_parameter("wvT", [HID, D], F32R, isOutput=False)
    woT = nc.declare_dram_parameter("woT", [G * D, HID], F32R, isOutput=False)
    cosT = nc.declare_dram_parameter("cosT", [D, S], F32, isOutput=False)
    sinT = nc.declare_dram_parameter("sinT", [D, S], F32, isOutput=False)
    permT = nc.declare_dram_parameter("permT", [D, D], F32R, isOutput=False)
    bias4 = nc.declare_dram_parameter("bias4", [4, 128, QCH], F32, isOutput=False)

    kT_out = nc.declare_dram_parameter("kT_out", [D, S], F32, isOutput=True)
    v_out = nc.declare_dram_parameter("v_out", [KV, D], F32, isOutput=True)
    o_out = nc.declare_dram_parameter("o_out", [S // N_CORES, HID], ODT, isOutput=True)

    from contextlib import ExitStack

    with tile.TileContext(nc) as tc, ExitStack() as ctx_stack:
        _body(
            nc, tc, ctx_stack, hT, wqT, wkT, wvT, woT, cosT, sinT, permT, bias4,
            kT_out, v_out, o_out,
        )
    nc.compile()
    return nc


def _body(
    nc, tc, ctx_stack, hT, wqT, wkT, wvT, woT, cosT, sinT, permT, bias4,
    kT_out, v_out, o_out,
):
    ec = ctx_stack.enter_context
    persist = ec(tc.tile_pool(name="persist", bufs=1))
    wbig = ec(tc.tile_pool(name="wbig", bufs=1))
    stream = ec(tc.tile_pool(name="stream", bufs=4))
    tmp = ec(tc.tile_pool(name="tmp", bufs=4))
    es_pool = ec(tc.tile_pool(name="es", bufs=3))
    ctx_pool = ec(tc.tile_pool(name="ctx", bufs=4))
    osb_pool = ec(tc.tile_pool(name="osb", bufs=2))
    dram = ec(tc.tile_pool(name="dram", bufs=1, space="DRAM"))

    # --- persistent SBUF ---
    ident = persist.tile([128, 128], F32)
    make_identity(nc, ident[:])
    permT_sb = persist.tile([128, 128], F32R)
    nc.gpsimd.dma_start(out=permT_sb[:], in_=permT[:])
    ones_f32 = persist.tile([128, 1], F32)
    nc.vector.memset(ones_f32[:], 1.0)
    ones_col = persist.tile([128, 1], F32R)
    nc.vector.tensor_copy(ones_col[:], ones_f32[:])
    onesr_f32 = persist.tile([1, 128], F32)
    nc.vector.memset(onesr_f32[:], 1.0)
    ones_row = persist.tile([1, 128], F32R)
    nc.vector.tensor_copy(ones_row[:], onesr_f32[:])

    wk_sb = persist.tile([128, HT * D], F32R)  # [p, (t d)]
    wv_sb = persist.tile([128, HT * D], F32R)
    bias_sb = persist.tile([128, 4 * QCH], F32)
    nc.gpsimd.dma_start(
        out=bias_sb[:].rearrange("p (t n) -> p t n", n=QCH),
        in_=bias4.rearrange("t p n -> p t n"),
    )

    # per-(head, chunk) / per-chunk tiles: Tile tracks dependencies at tile
    # granularity, so one big tile would make chunk-0 attention wait for the
    # chunk-3 RoPE epilogue.
    qT_c = [
        [persist.tile([128, QCH], F32R, name=f"qT_{h}_{j}") for j in range(NCH)]
        for h in range(G)
    ]
    kT_c = [persist.tile([128, QCH], F32R, name=f"kT_{j}") for j in range(NCH)]
    v_c = [persist.tile([128, QCH], F32R, name=f"v_{j}") for j in range(NCH)]

    # wq is only needed during the projection phase, wo afterwards; they share
    # one 8MB slot.  Per-h-tile slice DMAs, interleaved wq/wk/wv in t order,
    # so the t=0 matmuls start as soon as the first three slices land.
    wq_sb = wbig.tile([128, HT * G * D], F32R, tag="w")  # [p, (t m)]
    for t in range(HT):
        nc.gpsimd.dma_start(
            out=wq_sb[:, t * G * D : (t + 1) * G * D],
            in_=wqT[t * 128 : (t + 1) * 128, :],
        )
        nc.gpsimd.dma_start(
            out=wk_sb[:, t * D : (t + 1) * D], in_=wkT[t * 128 : (t + 1) * 128, :]
        )
        nc.gpsimd.dma_start(
            out=wv_sb[:, t * D : (t + 1) * D], in_=wvT[t * 128 : (t + 1) * 128, :]
        )

    # RS pieces: big early pieces overlap compute; small tail pieces cut the
    # exposed latency of the final collective.  One DRAM tile per piece —
    # a single big o_part tile would give later stores a false whole-tile
    # WAR dependency on each in-flight ReduceScatter.
    pieces = PIECES
    o_parts = [
        dram.tile([n, HID], ODT, tag=f"opart{i}", name=f"o_part{i}")
        for i, (r0, n) in enumerate(pieces)
    ]
    o_shards = [
        dram.tile([n // N_CORES, HID], ODT, tag=f"osh{i}", name=f"o_shard{i}")
        for i, (r0, n) in enumerate(pieces)
    ]

    def piece_of_row(row):
        for i, (r0, n) in enumerate(pieces):
            if r0 <= row < r0 + n:
                return i, r0
        raise AssertionError(row)

    # ---------------- Phase A: QKV projection + RoPE + V transpose ----------
    with tc.tile_pool(name="psA", bufs=1, space="PSUM") as psA:
        for j in range(NCH):
            q0 = QCH * j
            cos_sb = stream.tile([128, QCH], F32, tag="cs", bufs=2)
            sin_sb = stream.tile([128, QCH], F32, tag="cs", bufs=2)
            nc.gpsimd.dma_start(out=cos_sb[:], in_=cosT[:, q0 : q0 + QCH])
            nc.gpsimd.dma_start(out=sin_sb[:], in_=sinT[:, q0 : q0 + QCH])

            ps = [
                psA.tile([128, QCH], F32, tag="proj", name=f"proj{j}_{m}", bufs=6)
                for m in range(6)
            ]
            for t in range(HT):
                ht = stream.tile([128, QCH], F32R, tag="ht", bufs=6)
                nc.sync.dma_start(
                    out=ht[:], in_=hT[t * 128 : (t + 1) * 128, q0 : q0 + QCH]
                )
                fl = dict(start=(t == 0), stop=(t == HT - 1), skip_group_check=True)
                for m in range(G):
                    nc.tensor.matmul(
                        ps[m][:],
                        wq_sb[:, t * G * D + m * 128 : t * G * D + (m + 1) * 128],
                        ht[:],
                        **fl,
                    )
                nc.tensor.matmul(
                    ps[4][:], wk_sb[:, t * 128 : (t + 1) * 128], ht[:], **fl
                )
                nc.tensor.matmul(
                    ps[5][:], wv_sb[:, t * 128 : (t + 1) * 128], ht[:], **fl
                )

            # V first (its ACT copy unblocks the PE transposes that sit next
            # in the PE instruction stream), then RoPE for the 4 q heads + k.
            vt = tmp.tile([128, QCH], F32, tag="scratch")
            nc.scalar.copy(vt[:], ps[5][:])
            for b in range(QCH // 128):
                kvi = 4 * j + b
                pst = psA.tile([128, 128], F32, tag="tr", bufs=2)
                nc.tensor.transpose(pst[:], vt[:, b * 128 : (b + 1) * 128], ident[:])
                nc.vector.tensor_copy(v_c[j][:, b * 128 : (b + 1) * 128], pst[:])
                nc.scalar.dma_start(
                    out=v_out[kvi * 128 : (kvi + 1) * 128, :],
                    in_=v_c[j][:, b * 128 : (b + 1) * 128].bitcast(F32),
                )

            # RoPE for the 4 q heads and k: out = ps*cos + rot(ps)*sin_eff
            for m in range(5):
                src = ps[m] if m < G else ps[4]
                if m < G:
                    dst = qT_c[m][j][:]
                else:
                    dst = kT_c[j][:]
                # rotate-half via a PE permutation matmul (a partition-shift
                # SBUF->SBUF DMA here serializes ~2us per copy and delays the
                # PSUM release that gates phase B)
                qraw = tmp.tile([128, QCH], F32R, tag="scratch")
                if m % 2 == 0:
                    nc.scalar.copy(qraw[:], src[:])
                else:
                    nc.vector.tensor_copy(qraw[:], src[:])
                rot_ps = psA.tile([128, QCH], F32, tag="tr", bufs=2)
                nc.tensor.matmul(rot_ps[:], permT_sb[:], qraw[:], start=True, stop=True)
                t2 = tmp.tile([128, QCH], F32, tag="scratch")
                nc.vector.tensor_mul(t2[:], rot_ps[:], sin_sb[:])
                # in-place: qraw *= cos; dst = qraw + t2
                nc.vector.tensor_mul(qraw[:], qraw[:].bitcast(F32), cos_sb[:])
                nc.vector.tensor_add(dst, qraw[:].bitcast(F32), t2[:])
            nc.scalar.dma_start(
                out=kT_out[:, q0 : q0 + QCH], in_=kT_c[j][:].bitcast(F32)
            )

    # wo replaces wq in the shared slot; per-dh slices so o_proj dh=0 starts early
    wo_sb = wbig.tile([128, G * HID], F32R, tag="w")  # [p, (dh n)]
    for dh in range(G):
        nc.gpsimd.dma_start(
            out=wo_sb[:, dh * HID : (dh + 1) * HID],
            in_=woT[dh * 128 : (dh + 1) * 128, :],
        )

    # ---------------- Phase B: attention + o_proj + ReduceScatter -----------
    with tc.tile_pool(name="psB", bufs=1, space="PSUM") as psB:
        for j in range(NCH):
            q0 = QCH * j
            nkv = (q0 + QCH) // 128  # causal: kv tiles 0..nkv-1
            ctxs = [None] * G
            pending = None  # (ctx_ps, den_ps, h) awaiting normalize

            def normalize(ctx_ps, den_ps, h):
                # deferred by one head so the bc matmul (which waits on the
                # DVE reciprocal) doesn't stall the in-order PE stream
                inv = tmp.tile([1, QCH], F32R, tag="inv", bufs=2)
                with nc.allow_low_precision(reason="f32r softmax denom"):
                    nc.vector.reciprocal(inv[:], den_ps[:])
                bc_ps = psB.tile([128, QCH], F32, tag="s", bufs=4)
                nc.tensor.matmul(bc_ps[:], ones_row[:], inv[:], start=True, stop=True)
                ctxc = tmp.tile([128, QCH], F32, tag="scratch")
                nc.scalar.copy(ctxc[:], ctx_ps[:])
                ctx_sb = ctx_pool.tile([128, QCH], F32R, tag="ctx_sb")
                nc.vector.tensor_mul(ctx_sb[:], ctxc[:], bc_ps[:])
                ctxs[h] = ctx_sb

            for h in range(G):
                ctx_ps = psB.tile([128, QCH], F32, tag="ctx", bufs=2)
                den_ps = psB.tile([1, QCH], F32, tag="den", bufs=2)
                for kv in range(nkv):
                    jc, b = kv // 4, kv % 4
                    s_ps = psB.tile([128, QCH], F32, tag="s", bufs=4)
                    nc.tensor.matmul(
                        s_ps[:],
                        kT_c[jc][:, b * 128 : (b + 1) * 128],
                        qT_c[h][j][:],
                        start=True,
                        stop=True,
                    )
                    db = kv - (nkv - 4)
                    if db >= 0:  # diagonal band: apply mask bias
                        nc.vector.tensor_add(
                            s_ps[:], s_ps[:], bias_sb[:, db * QCH : (db + 1) * QCH]
                        )
                    es = es_pool.tile([128, QCH], F32R, tag="es")
                    nc.scalar.activation(
                        es[:], s_ps[:], mybir.ActivationFunctionType.Exp
                    )
                    flk = dict(
                        start=(kv == 0), stop=(kv == nkv - 1), skip_group_check=True
                    )
                    nc.tensor.matmul(
                        ctx_ps[:], v_c[jc][:, b * 128 : (b + 1) * 128], es[:], **flk
                    )
                    nc.tensor.matmul(den_ps[:], ones_col[:], es[:], **flk)
                if pending is not None:
                    normalize(*pending)
                pending = (ctx_ps, den_ps, h)
            normalize(*pending)

            for qt in range(QCH // 128):
                for nh in range(HID // QCH):
                    o_ps = psB.tile([128, QCH], F32, tag="s", bufs=4)
                    for dh in range(G):
                        nc.tensor.matmul(
                            o_ps[:],
                            ctxs[dh][:, qt * 128 : (qt + 1) * 128],
                            wo_sb[:, dh * HID + nh * QCH : dh * HID + (nh + 1) * QCH],
                            start=(dh == 0),
                            stop=(dh == G - 1),
                            skip_group_check=True,
                        )
                    o_sb = osb_pool.tile([128, QCH], ODT, tag="osb")
                    nc.vector.tensor_copy(o_sb[:], o_ps[:])
                    row = q0 + qt * 128
                    pi, pr0 = piece_of_row(row)
                    nc.scalar.dma_start(
                        out=o_parts[pi][
                            row - pr0 : row - pr0 + 128, nh * QCH : (nh + 1) * QCH
                        ],
                        in_=o_sb[:],
                    )
                # fire the RS for any piece whose rows are now fully written
                row_end = q0 + (qt + 1) * 128
                for i, (r0, n) in enumerate(pieces):
                    if r0 + n == row_end:
                        nc.gpsimd.collective_compute(
                            "ReduceScatter",
                            mybir.AluOpType.add,
                            replica_groups=[list(range(N_CORES))],
                            ins=[o_parts[i][:]],
                            outs=[o_shards[i][:]],
                        )
                        # sync queue: a store waiting on the RS would
                        # head-of-line-block the scalar queue's o_part stores
                        nc.sync.dma_start(
                            out=o_out[r0 // N_CORES : (r0 + n) // N_CORES, :],
                            in_=o_shards[i][:],
                        )


def _prep_inputs(hidden_states, attention_mask, cos, sin, Wq, Wk, Wv, Wo):
    h = np.ascontiguousarray(np.asarray(hidden_states, np.float32).reshape(S, HID))
    hT = np.ascontiguousarray(h.T)
    cos2 = np.asarray(cos, np.float32).reshape(S, D)
    sin2 = np.asarray(sin, np.float32).reshape(S, D)
    cosT = np.ascontiguousarray(cos2.T)
    sgn = np.where(np.arange(D) < D // 2, -1.0, 1.0).astype(np.float32)
    sinT = np.ascontiguousarray((sin2 * sgn).T)

    permM = np.zeros((D, D), np.float32)
    for dcol in range(D):
        permM[(dcol + 64) % D, dcol] = 1.0  # lhsT of the rotate-half permutation

    mask2 = np.asarray(attention_mask, np.float32).reshape(S, KV)
    # The kernel hardcodes the causal block structure; verify it holds.
    expect = np.tril(np.ones((S, KV), np.float32))
    if not np.array_equal(mask2, expect):
        raise ValueError("kernel compiled for a causal (tril) attention_mask")
    bias4 = np.empty((4, 128, QCH), np.float32)
    for t in range(4):
        sub = mask2[0:QCH, t * 128 : (t + 1) * 128]  # [q, kv_local]
        bias4[t] = np.where(sub.T > 0.5, 0.0, NEG).astype(np.float32)

    scale = 1.0 / math.sqrt(D)
    Wq = np.asarray(Wq, np.float32)
    Wk = np.asarray(Wk, np.float32)
    Wv = np.asarray(Wv, np.float32)
    Wo = np.asarray(Wo, np.float32)

    in_maps = []
    for c in range(N_CORES):
        wq_c = np.ascontiguousarray((Wq[c * G * D : (c + 1) * G * D, :] * scale).T)
        wk_c = np.ascontiguousarray(Wk[c * D : (c + 1) * D, :].T)
        wv_c = np.ascontiguousarray(Wv[c * D : (c + 1) * D, :].T)
        wo_c = np.ascontiguousarray(Wo[:, c * G * D : (c + 1) * G * D].T)
        in_maps.append(
            dict(
                hT=hT,
                wqT=wq_c,
                wkT=wk_c,
                wvT=wv_c,
                woT=wo_c,
                cosT=cosT,
                sinT=sinT,
                permT=permM,
                bias4=bias4,
            )
        )
    return in_maps


def kernel(
    hidden_states,
    attention_mask,
    cos,
    sin,
    past_key,
    past_value,
    Wq,
    Wk,
    Wv,
    Wo,
    seq_positions,
    batch_position,
):
    global _compiled, LAST_RESULT
    assert int(np.asarray(seq_positions).reshape(-1)[0]) == 0
    assert int(np.asarray(batch_position)) == 0

    if _compiled is None:
        _compiled = _build()
    nc = _compiled

    in_maps = _prep_inputs(hidden_states, attention_mask, cos, sin, Wq, Wk, Wv, Wo)
    res = run_bass_kernel_spmd(nc, in_maps, list(range(N_CORES)), trace=TRACE)
    LAST_RESULT = res

    pieces = PIECES
    key_cache = np.empty((1, KVH, KV, D), np.float32)
    value_cache = np.empty((1, KVH, KV, D), np.float32)
    attn_out = np.empty((S, HID), np.float32)
    for c in range(N_CORES):
        r = res.results[c]
        key_cache[0, c] = r["kT_out"].T
        value_cache[0, c] = r["v_out"]
        o_np = np.asarray(r["o_out"], np.float32)
        for r0, n in pieces:
            sh = n // N_CORES
            attn_out[r0 + sh * c : r0 + sh * (c + 1)] = o_np[
                r0 // N_CORES : r0 // N_CORES + sh
            ]
    return attn_out.reshape(1, S, HID), key_cache, value_cache


# revision 78
# speedup vs baseline: 1.3677x; 1.0034x over previous
"""Trainium2 Bass kernel for decoder-only GQA attention (tensor-parallel x8).

Problem (hardcoded): B=1, S=2048, HID=4096, H=32 q-heads, KVH=8 kv-heads,
D=128, KV_LEN=2048, seq_position=0, batch_position=0, causal mask.

Sharding: tensor-parallel over the 8 kv heads.  Core c owns kv head c and
q heads 4c..4c+3.  Wq/Wk/Wv sharded along their output (head) dim, Wo along
its input dim.  Each core computes a partial o_proj output [2048, 4096];
a per-chunk ReduceScatter sums the partials and leaves row-shard slices
that the host reassembles (the "gather" half of the hinted all-reduce is
done by the host-side unshard).

Device-side dataflow is entirely "transposed" to keep every matmul
transpose-free:
  hiddenT [hid, s] -> QT/KT [d, s] (RoPE applied in the transposed layout
  via a partition-rotation DMA and sign-folded sin), VT -> V via PE
  transpose, scoresT [kv, q] = K @ QT, softmax along the partition (kv)
  axis with the denominator computed by a ones-column matmul, ctxT [d, q]
  = V.T @ expST, o [q, hid] = ctxT.T @ WoT.

Matmuls run in float32r (fp32 with reduced mantissa, 4x the fp32 rate,
~1e-4 matmul error); everything else is fp32.
"""

import math

import numpy as np

import concourse.bacc as bacc
import concourse.mybir as mybir
import concourse.tile as tile
from concourse.bass_utils import run_bass_kernel_spmd
from concourse.masks import make_identity

S = 2048
HID = 4096
H = 32
KVH = 8
D = 128
G = H // KVH  # q heads per core
KV = 2048
N_CORES = 8
QCH = 512  # q-rows per chunk
NCH = S // QCH  # 4 chunks
HT = HID // 128  # 32 h-tiles
NEG = -1.0e9

F32 = mybir.dt.float32
F32R = mybir.dt.float32r
BF16 = mybir.dt.bfloat16

# Set by test.py to collect HW timing/profiles.
TRACE = False
LAST_RESULT = None

# ReduceScatter pieces (row ranges of attn_out), shared by device and host code.
# Smaller tail pieces shrink the exposed latency of the final collective.
PIECES = [
    (0, 512), (512, 512), (1024, 512), (1536, 256), (1792, 256),
]
# bf16 partial-sums for the o_proj ReduceScatter: halves collective traffic
# (the mesh bursts starve app DMA) at ~2e-3 attn_out error.
RS_BF16 = True
ODT = BF16 if RS_BF16 else F32

_compiled = None


def _build():
    nc = bacc.Bacc("TRN2", target_bir_lowering=False, num_devices=N_CORES)

    hT = nc.declare_dram_parameter("hT", [HID, S], F32R, isOutput=False)
    wqT = nc.declare_dram_parameter("wqT", [HID, G * D], F32R, isOutput=False)
    wkT = nc.declare_dram_parameter("wkT", [HID, D], F32R, isOutput=False)
    wvT = nc.declare_dram_parameter("wvT", [HID, D], F32R, isOutput=False)
    woT = nc.declare_dram_parameter("woT", [G * D, HID], F32R, isOutput=False)
    cosT = nc.declare_dram_parameter("cosT", [D, S], F32, isOutput=False)
    sinT = nc.declare_dram_parameter("sinT", [D, S], F32, isOutput=False)
    permT = nc.declare_dram_parameter("permT", [D, D], F32R, isOutput=False)
    bias4 = nc.declare_dram_parameter("bias4", [4, 128, QCH], F32, isOutput=False)

    kT_out = nc.declare_dram_parameter("kT_out", [D, S], F32, isOutput=True)
    v_out = nc.declare_dram_parameter("v_out", [KV, D], F32, isOutput=True)
    o_out = nc.declare_dram_parameter("o_out", [S // N_CORES, HID], ODT, isOutput=True)

    from contextlib import ExitStack

    with tile.TileContext(nc) as tc, ExitStack() as ctx_stack:
        _body(
            nc, tc, ctx_stack, hT, wqT, wkT, wvT, woT, cosT, sinT, permT, bias4,
            kT_out, v_out, o_out,
        )
    nc.compile()
    return nc


def _body(
    nc, tc, ctx_stack, hT, wqT, wkT, wvT, woT, cosT, sinT, permT, bias4,
    kT_out, v_out, o_out,
):
    ec = ctx_stack.enter_context
    persist = ec(tc.tile_pool(name="persist", bufs=1))
    wbig = ec(tc.tile_pool(name="wbig", bufs=1))
    stream = ec(tc.tile_pool(name="stream", bufs=4))
    tmp = ec(tc.tile_pool(name="tmp", bufs=4))
    es_pool = ec(tc.tile_pool(name="es", bufs=3))
    ctx_pool = ec(tc.tile_pool(name="ctx", bufs=4))
    osb_pool = ec(tc.tile_pool(name="osb", bufs=2))
    dram = ec(tc.tile_pool(name="dram", bufs=1, space="DRAM"))

    # --- persistent SBUF ---
    ident = persist.tile([128, 128], F32)
    make_identity(nc, ident[:])
    ones_f32 = persist.tile([128, 1], F32)
    nc.vector.memset(ones_f32[:], 1.0)
    ones_col = persist.tile([128, 1], F32R)
    nc.vector.tensor_copy(ones_col[:], ones_f32[:])
    onesr_f32 = persist.tile([1, 128], F32)
    nc.vector.memset(onesr_f32[:], 1.0)
    ones_row = persist.tile([1, 128], F32R)
    nc.vector.tensor_copy(ones_row[:], onesr_f32[:])

    wk_sb = persist.tile([128, HT * D], F32R)  # [p, (t d)]
    wv_sb = persist.tile([128, HT * D], F32R)
    bias_sb = persist.tile([128, 4 * QCH], F32)

    # per-(head, chunk) / per-chunk tiles: Tile tracks dependencies at tile
    # granularity, so one big tile would make chunk-0 attention wait for the
    # chunk-3 RoPE epilogue.
    qT_c = [
        [persist.tile([128, QCH], F32R, name=f"qT_{h}_{j}") for j in range(NCH)]
        for h in range(G)
    ]
    kT_c = [persist.tile([128, QCH], F32R, name=f"kT_{j}") for j in range(NCH)]
    v_c = [persist.tile([128, QCH], F32R, name=f"v_{j}") for j in range(NCH)]

    # wq is only needed during the projection phase, wo afterwards; they share
    # one 8MB slot.  Per-h-tile slice DMAs, interleaved wq/wk/wv in t order,
    # so the t=0 matmuls start as soon as the first three slices land.
    wq_sb = wbig.tile([128, HT * G * D], F32R, tag="w")  # [p, (t m)]
    for t in range(HT):
        nc.gpsimd.dma_start(
            out=wq_sb[:, t * G * D : (t + 1) * G * D],
            in_=wqT[t * 128 : (t + 1) * 128, :],
        )
        nc.gpsimd.dma_start(
            out=wk_sb[:, t * D : (t + 1) * D], in_=wkT[t * 128 : (t + 1) * 128, :]
        )
        nc.gpsimd.dma_start(
            out=wv_sb[:, t * D : (t + 1) * D], in_=wvT[t * 128 : (t + 1) * 128, :]
        )
    # constants after the weight slices: the t=0 matmuls gate kernel start
    permT_sb = persist.tile([128, 128], F32R)
    nc.gpsimd.dma_start(out=permT_sb[:], in_=permT[:])
    nc.gpsimd.dma_start(
        out=bias_sb[:].rearrange("p (t n) -> p t n", n=QCH),
        in_=bias4.rearrange("t p n -> p t n"),
    )

    # RS pieces: big early pieces overlap compute; small tail pieces cut the
    # exposed latency of the final collective.  One DRAM tile per piece —
    # a single big o_part tile would give later stores a false whole-tile
    # WAR dependency on each in-flight ReduceScatter.
    pieces = PIECES
    o_parts = [
        dram.tile([n, HID], ODT, tag=f"opart{i}", name=f"o_part{i}")
        for i, (r0, n) in enumerate(pieces)
    ]
    o_shards = [
        dram.tile([n // N_CORES, HID], ODT, tag=f"osh{i}", name=f"o_shard{i}")
        for i, (r0, n) in enumerate(pieces)
    ]

    def piece_of_row(row):
        for i, (r0, n) in enumerate(pieces):
            if r0 <= row < r0 + n:
                return i, r0
        raise AssertionError(row)

    # ---------------- Phase A: QKV projection + RoPE + V transpose ----------
    with tc.tile_pool(name="psA", bufs=1, space="PSUM") as psA:
        for j in range(NCH):
            q0 = QCH * j
            cos_sb = stream.tile([128, QCH], F32, tag="cs", bufs=2)
            sin_sb = stream.tile([128, QCH], F32, tag="cs", bufs=2)
            nc.gpsimd.dma_start(out=cos_sb[:], in_=cosT[:, q0 : q0 + QCH])
            nc.gpsimd.dma_start(out=sin_sb[:], in_=sinT[:, q0 : q0 + QCH])

            ps = [
                psA.tile([128, QCH], F32, tag="proj", name=f"proj{j}_{m}", bufs=6)
                for m in range(6)
            ]
            for t in range(HT):
                ht = stream.tile([128, QCH], F32R, tag="ht", bufs=6)
                nc.sync.dma_start(
                    out=ht[:], in_=hT[t * 128 : (t + 1) * 128, q0 : q0 + QCH]
                )
                fl = dict(start=(t == 0), stop=(t == HT - 1), skip_group_check=True)
                for m in range(G):
                    nc.tensor.matmul(
                        ps[m][:],
                        wq_sb[:, t * G * D + m * 128 : t * G * D + (m + 1) * 128],
                        ht[:],
                        **fl,
                    )
                nc.tensor.matmul(
                    ps[4][:], wk_sb[:, t * 128 : (t + 1) * 128], ht[:], **fl
                )
                nc.tensor.matmul(
                    ps[5][:], wv_sb[:, t * 128 : (t + 1) * 128], ht[:], **fl
                )

            # V first (its ACT copy unblocks the PE transposes that sit next
            # in the PE instruction stream), then RoPE for the 4 q heads + k.
            vt = tmp.tile([128, QCH], F32, tag="scratch")
            nc.scalar.copy(vt[:], ps[5][:])
            for b in range(QCH // 128):
                kvi = 4 * j + b
                pst = psA.tile([128, 128], F32, tag="tr", bufs=2)
                nc.tensor.transpose(pst[:], vt[:, b * 128 : (b + 1) * 128], ident[:])
                nc.vector.tensor_copy(v_c[j][:, b * 128 : (b + 1) * 128], pst[:])
                nc.scalar.dma_start(
                    out=v_out[kvi * 128 : (kvi + 1) * 128, :],
                    in_=v_c[j][:, b * 128 : (b + 1) * 128].bitcast(F32),
                )

            # RoPE for the 4 q heads and k: out = ps*cos + rot(ps)*sin_eff
            for m in range(5):
                src = ps[m] if m < G else ps[4]
                if m < G:
                    dst = qT_c[m][j][:]
                else:
                    dst = kT_c[j][:]
                # rotate-half via a PE permutation matmul (a partition-shift
                # SBUF->SBUF DMA here serializes ~2us per copy and delays the
                # PSUM release that gates phase B)
                qraw = tmp.tile([128, QCH], F32R, tag="scratch")
                if m % 2 == 0:
                    nc.scalar.copy(qraw[:], src[:])
                else:
                    nc.vector.tensor_copy(qraw[:], src[:])
                rot_ps = psA.tile([128, QCH], F32, tag="tr", bufs=2)
                nc.tensor.matmul(rot_ps[:], permT_sb[:], qraw[:], start=True, stop=True)
                t2 = tmp.tile([128, QCH], F32, tag="scratch")
                nc.vector.tensor_mul(t2[:], rot_ps[:], sin_sb[:])
                # in-place: qraw *= cos; dst = qraw + t2
                nc.vector.tensor_mul(qraw[:], qraw[:].bitcast(F32), cos_sb[:])
                nc.vector.tensor_add(dst, qraw[:].bitcast(F32), t2[:])
            nc.scalar.dma_start(
                out=kT_out[:, q0 : q0 + QCH], in_=kT_c[j][:].bitcast(F32)
            )

    # wo replaces wq in the shared slot; per-dh slices so o_proj dh=0 starts early
    wo_sb = wbig.tile([128, G * HID], F32R, tag="w")  # [p, (dh n)]
    for dh in range(G):
        nc.gpsimd.dma_start(
            out=wo_sb[:, dh * HID : (dh + 1) * HID],
            in_=woT[dh * 128 : (dh + 1) * 128, :],
        )

    # ---------------- Phase B: attention + o_proj + ReduceScatter -----------
    with tc.tile_pool(name="psB", bufs=1, space="PSUM") as psB:
        for j in range(NCH):
            q0 = QCH * j
            nkv = (q0 + QCH) // 128  # causal: kv tiles 0..nkv-1
            ctxs = [None] * G
            pending = None  # (ctx_ps, den_ps, h) awaiting normalize

            def normalize(ctx_ps, den_ps, h):
                # deferred by one head so the bc matmul (which waits on the
                # DVE reciprocal) doesn't stall the in-order PE stream
                inv = tmp.tile([1, QCH], F32R, tag="inv", bufs=2)
                with nc.allow_low_precision(reason="f32r softmax denom"):
                    nc.vector.reciprocal(inv[:], den_ps[:])
                bc_ps = psB.tile([128, QCH], F32, tag="s", bufs=4)
                nc.tensor.matmul(bc_ps[:], ones_row[:], inv[:], start=True, stop=True)
                ctxc = tmp.tile([128, QCH], F32, tag="scratch")
                nc.scalar.copy(ctxc[:], ctx_ps[:])
                ctx_sb = ctx_pool.tile([128, QCH], F32R, tag="ctx_sb")
                nc.vector.tensor_mul(ctx_sb[:], ctxc[:], bc_ps[:])
                ctxs[h] = ctx_sb

            for h in range(G):
                ctx_ps = psB.tile([128, QCH], F32, tag="ctx", bufs=2)
                den_ps = psB.tile([1, QCH], F32, tag="den", bufs=2)
                for kv in range(nkv):
                    jc, b = kv // 4, kv % 4
                    s_ps = psB.tile([128, QCH], F32, tag="s", bufs=4)
                    nc.tensor.matmul(
                        s_ps[:],
                        kT_c[jc][:, b * 128 : (b + 1) * 128],
                        qT_c[h][j][:],
                        start=True,
                        stop=True,
                    )
                    db = kv - (nkv - 4)
                    if db >= 0:  # diagonal band: apply mask bias
                        nc.vector.tensor_add(
                            s_ps[:], s_ps[:], bias_sb[:, db * QCH : (db + 1) * QCH]
                        )
                    es = es_pool.tile([128, QCH], F32R, tag="es")
                    nc.scalar.activation(
                        es[:], s_ps[:], mybir.ActivationFunctionType.Exp
                    )
                    flk = dict(
                        start=(kv == 0), stop=(kv == nkv - 1), skip_group_check=True
                    )
                    nc.tensor.matmul(
                        ctx_ps[:], v_c[jc][:, b * 128 : (b + 1) * 128], es[:], **flk
                    )
                    nc.tensor.matmul(den_ps[:], ones_col[:], es[:], **flk)
                if pending is not None:
                    normalize(*pending)
                pending = (ctx_ps, den_ps, h)
            normalize(*pending)

            for qt in range(QCH // 128):
                for nh in range(HID // QCH):
                    o_ps = psB.tile([128, QCH], F32, tag="s", bufs=4)
                    for dh in range(G):
                        nc.tensor.matmul(
                            o_ps[:],
                            ctxs[dh][:, qt * 128 : (qt + 1) * 128],
                            wo_sb[:, dh * HID + nh * QCH : dh * HID + (nh + 1) * QCH],
                            start=(dh == 0),
                            stop=(dh == G - 1),
                            skip_group_check=True,
                        )
                    o_sb = osb_pool.tile([128, QCH], ODT, tag="osb")
                    nc.vector.tensor_copy(o_sb[:], o_ps[:])
                    row = q0 + qt * 128
                    pi, pr0 = piece_of_row(row)
                    nc.scalar.dma_start(
                        out=o_parts[pi][
                            row - pr0 : row - pr0 + 128, nh * QCH : (nh + 1) * QCH
                        ],
                        in_=o_sb[:],
                    )
                # fire the RS for any piece whose rows are now fully written
                row_end = q0 + (qt + 1) * 128
                for i, (r0, n) in enumerate(pieces):
                    if r0 + n == row_end:
                        nc.gpsimd.collective_compute(
                            "ReduceScatter",
                            mybir.AluOpType.add,
                            replica_groups=[list(range(N_CORES))],
                            ins=[o_parts[i][:]],
                            outs=[o_shards[i][:]],
                        )
                        # sync queue: a store waiting on the RS would
                        # head-of-line-block the scalar queue's o_part stores
                        nc.sync.dma_start(
                            out=o_out[r0 // N_CORES : (r0 + n) // N_CORES, :],
                            in_=o_shards[i][:],
                        )


def _prep_inputs(hidden_states, attention_mask, cos, sin, Wq, Wk, Wv, Wo):
    h = np.ascontiguousarray(np.asarray(hidden_states, np.float32).reshape(S, HID))
    hT = np.ascontiguousarray(h.T)
    cos2 = np.asarray(cos, np.float32).reshape(S, D)
    sin2 = np.asarray(sin, np.float32).reshape(S, D)
    cosT = np.ascontiguousarray(cos2.T)
    sgn = np.where(np.arange(D) < D // 2, -1.0, 1.0).astype(np.float32)
    sinT = np.ascontiguousarray((sin2 * sgn).T)

    permM = np.zeros((D, D), np.float32)
    for dcol in range(D):
        permM[(dcol + 64) % D, dcol] = 1.0  # lhsT of the rotate-half permutation

    mask2 = np.asarray(attention_mask, np.float32).reshape(S, KV)
    # The kernel hardcodes the causal block structure; verify it holds.
    expect = np.tril(np.ones((S, KV), np.float32))
    if not np.array_equal(mask2, expect):
        raise ValueError("kernel compiled for a causal (tril) attention_mask")
    bias4 = np.empty((4, 128, QCH), np.float32)
    for t in range(4):
        sub = mask2[0:QCH, t * 128 : (t + 1) * 128]  # [q, kv_local]
        bias4[t] = np.where(sub.T > 0.5, 0.0, NEG).astype(np.float32)

    scale = 1.0 / math.sqrt(D)
    Wq = np.asarray(Wq, np.float32)
    Wk = np.asarray(Wk, np.float32)
    Wv = np.asarray(Wv, np.float32)
    Wo = np.asarray(Wo, np.float32)

    in_maps = []
    for c in range(N_CORES):
        wq_c = np.ascontiguousarray((Wq[c * G * D : (c + 1) * G * D, :] * scale).T)
        wk_c = np.ascontiguousarray(Wk[c * D : (c + 1) * D, :].T)
        wv_c = np.ascontiguousarray(Wv[c * D : (c + 1) * D, :].T)
        wo_c = np.ascontiguousarray(Wo[:, c * G * D : (c + 1) * G * D].T)
        in_maps.append(
            dict(
                hT=hT,
                wqT=wq_c,
                wkT=wk_c,
                wvT=wv_c,
                woT=wo_c,
                cosT=cosT,
                sinT=sinT,
                permT=permM,
                bias4=bias4,
            )
        )
    return in_maps


def kernel(
    hidden_states,
    attention_mask,
    cos,
    sin,
    past_key,
    past_value,
    Wq,
    Wk,
    Wv,
    Wo,
    seq_positions,
    batch_position,
):
    global _compiled, LAST_RESULT
    assert int(np.asarray(seq_positions).reshape(-1)[0]) == 0
    assert int(np.asarray(batch_position)) == 0

    if _compiled is None:
        _compiled = _build()
    nc = _compiled

    in_maps = _prep_inputs(hidden_states, attention_mask, cos, sin, Wq, Wk, Wv, Wo)
    res = run_bass_kernel_spmd(nc, in_maps, list(range(N_CORES)), trace=TRACE)
    LAST_RESULT = res

    pieces = PIECES
    key_cache = np.empty((1, KVH, KV, D), np.float32)
    value_cache = np.empty((1, KVH, KV, D), np.float32)
    attn_out = np.empty((S, HID), np.float32)
    for c in range(N_CORES):
        r = res.results[c]
        key_cache[0, c] = r["kT_out"].T
        value_cache[0, c] = r["v_out"]
        o_np = np.asarray(r["o_out"], np.float32)
        for r0, n in pieces:
            sh = n // N_CORES
            attn_out[r0 + sh * c : r0 + sh * (c + 1)] = o_np[
                r0 // N_CORES : r0 // N_CORES + sh
            ]
    return attn_out.reshape(1, S, HID), key_cache, value_cache


# revision 79
# speedup vs baseline: 1.5407x; 1.1266x over previous
"""Trainium2 Bass kernel for decoder-only GQA attention (tensor-parallel x8).

Problem (hardcoded): B=1, S=2048, HID=4096, H=32 q-heads, KVH=8 kv-heads,
D=128, KV_LEN=2048, seq_position=0, batch_position=0, causal mask.

Sharding: tensor-parallel over the 8 kv heads.  Core c owns kv head c and
q heads 4c..4c+3.  Wq/Wk/Wv sharded along their output (head) dim, Wo along
its input dim.  Each core computes a partial o_proj output [2048, 4096];
a per-chunk ReduceScatter sums the partials and leaves row-shard slices
that the host reassembles (the "gather" half of the hinted all-reduce is
done by the host-side unshard).

Device-side dataflow is entirely "transposed" to keep every matmul
transpose-free:
  hiddenT [hid, s] -> QT/KT [d, s] (RoPE applied in the transposed layout
  via a partition-rotation DMA and sign-folded sin), VT -> V via PE
  transpose, scoresT [kv, q] = K @ QT, softmax along the partition (kv)
  axis with the denominator computed by a ones-column matmul, ctxT [d, q]
  = V.T @ expST, o [q, hid] = ctxT.T @ WoT.

Matmuls run in float32r (fp32 with reduced mantissa, 4x the fp32 rate,
~1e-4 matmul error); everything else is fp32.
"""

import math

import numpy as np

import concourse.bacc as bacc
import concourse.mybir as mybir
import concourse.tile as tile
from concourse.bass_utils import run_bass_kernel_spmd
from concourse.masks import make_identity

S = 2048
HID = 4096
H = 32
KVH = 8
D = 128
G = H // KVH  # q heads per core
KV = 2048
N_CORES = 8
QCH = 512  # q-rows per chunk
NCH = S // QCH  # 4 chunks
HT = HID // 128  # 32 h-tiles
NEG = -1.0e9

F32 = mybir.dt.float32
F32R = mybir.dt.float32r
BF16 = mybir.dt.bfloat16

# Set by test.py to collect HW timing/profiles.
TRACE = False
LAST_RESULT = None

# ReduceScatter pieces (row ranges of attn_out), shared by device and host code.
# Smaller tail pieces shrink the exposed latency of the final collective.
PIECES = [
    (0, 512), (512, 512), (1024, 512), (1536, 256), (1792, 256),
]
# bf16 partial-sums for the o_proj ReduceScatter: halves collective traffic
# (the mesh bursts starve app DMA) at ~2e-3 attn_out error.
RS_BF16 = True
ODT = BF16 if RS_BF16 else F32

_compiled = None


def _build():
    nc = bacc.Bacc("TRN2", target_bir_lowering=False, num_devices=N_CORES)

    hT = nc.declare_dram_parameter("hT", [HID, S], F32R, isOutput=False)
    wqT = nc.declare_dram_parameter("wqT", [HID, G * D], F32R, isOutput=False)
    wkT = nc.declare_dram_parameter("wkT", [HID, D], F32R, isOutput=False)
    wvT = nc.declare_dram_parameter("wvT", [HID, D], F32R, isOutput=False)
    woT = nc.declare_dram_parameter("woT", [G * D, HID], F32R, isOutput=False)
    cosT = nc.declare_dram_parameter("cosT", [D, S], F32, isOutput=False)
    sinT = nc.declare_dram_parameter("sinT", [D, S], F32, isOutput=False)
    permT = nc.declare_dram_parameter("permT", [D, D], F32R, isOutput=False)
    bias4 = nc.declare_dram_parameter("bias4", [4, 128, QCH], F32, isOutput=False)

    kT_out = nc.declare_dram_parameter("kT_out", [D, S], F32, isOutput=True)
    v_out = nc.declare_dram_parameter("v_out", [KV, D], F32, isOutput=True)
    o_out = nc.declare_dram_parameter("o_out", [S // N_CORES, HID], ODT, isOutput=True)

    from contextlib import ExitStack

    with tile.TileContext(nc) as tc, ExitStack() as ctx_stack:
        _body(
            nc, tc, ctx_stack, hT, wqT, wkT, wvT, woT, cosT, sinT, permT, bias4,
            kT_out, v_out, o_out,
        )
    nc.compile()
    return nc


def _body(
    nc, tc, ctx_stack, hT, wqT, wkT, wvT, woT, cosT, sinT, permT, bias4,
    kT_out, v_out, o_out,
):
    ec = ctx_stack.enter_context
    persist = ec(tc.tile_pool(name="persist", bufs=1))
    wbig = ec(tc.tile_pool(name="wbig", bufs=1))
    stream = ec(tc.tile_pool(name="stream", bufs=4))
    tmp = ec(tc.tile_pool(name="tmp", bufs=4))
    es_pool = ec(tc.tile_pool(name="es", bufs=3))
    ctx_pool = ec(tc.tile_pool(name="ctx", bufs=4))
    osb_pool = ec(tc.tile_pool(name="osb", bufs=2))
    dram = ec(tc.tile_pool(name="dram", bufs=1, space="DRAM"))

    # --- persistent SBUF ---
    ident = persist.tile([128, 128], F32)
    make_identity(nc, ident[:])
    ones_f32 = persist.tile([128, 1], F32)
    nc.vector.memset(ones_f32[:], 1.0)
    ones_col = persist.tile([128, 1], F32R)
    nc.vector.tensor_copy(ones_col[:], ones_f32[:])
    onesr_f32 = persist.tile([1, 128], F32)
    nc.vector.memset(onesr_f32[:], 1.0)
    ones_row = persist.tile([1, 128], F32R)
    nc.vector.tensor_copy(ones_row[:], onesr_f32[:])

    wk_sb = persist.tile([128, HT * D], F32R)  # [p, (t d)]
    wv_sb = persist.tile([128, HT * D], F32R)
    bias_sb = persist.tile([128, 4 * QCH], F32)

    # per-(head, chunk) / per-chunk tiles: Tile tracks dependencies at tile
    # granularity, so one big tile would make chunk-0 attention wait for the
    # chunk-3 RoPE epilogue.
    qT_c = [
        [persist.tile([128, QCH], F32R, name=f"qT_{h}_{j}") for j in range(NCH)]
        for h in range(G)
    ]
    kT_c = [persist.tile([128, QCH], F32R, name=f"kT_{j}") for j in range(NCH)]
    v_c = [persist.tile([128, QCH], F32R, name=f"v_{j}") for j in range(NCH)]

    # wq is only needed during the projection phase, wo afterwards; they share
    # one 8MB slot.  Per-h-tile slice DMAs, interleaved wq/wk/wv in t order,
    # so the t=0 matmuls start as soon as the first three slices land.
    wq_sb = wbig.tile([128, HT * G * D], F32R, tag="w")  # [p, (t m)]
    for t in range(HT):
        nc.gpsimd.dma_start(
            out=wq_sb[:, t * G * D : (t + 1) * G * D],
            in_=wqT[t * 128 : (t + 1) * 128, :],
        )
        nc.gpsimd.dma_start(
            out=wk_sb[:, t * D : (t + 1) * D], in_=wkT[t * 128 : (t + 1) * 128, :]
        )
        nc.gpsimd.dma_start(
            out=wv_sb[:, t * D : (t + 1) * D], in_=wvT[t * 128 : (t + 1) * 128, :]
        )
    # constants after the weight slices: the t=0 matmuls gate kernel start
    permT_sb = persist.tile([128, 128], F32R)
    nc.gpsimd.dma_start(out=permT_sb[:], in_=permT[:])
    nc.gpsimd.dma_start(
        out=bias_sb[:].rearrange("p (t n) -> p t n", n=QCH),
        in_=bias4.rearrange("t p n -> p t n"),
    )

    # RS pieces: big early pieces overlap compute; small tail pieces cut the
    # exposed latency of the final collective.  One DRAM tile per piece —
    # a single big o_part tile would give later stores a false whole-tile
    # WAR dependency on each in-flight ReduceScatter.
    pieces = PIECES
    o_parts = [
        dram.tile([n, HID], ODT, tag=f"opart{i}", name=f"o_part{i}")
        for i, (r0, n) in enumerate(pieces)
    ]
    o_shards = [
        dram.tile([n // N_CORES, HID], ODT, tag=f"osh{i}", name=f"o_shard{i}")
        for i, (r0, n) in enumerate(pieces)
    ]

    def piece_of_row(row):
        for i, (r0, n) in enumerate(pieces):
            if r0 <= row < r0 + n:
                return i, r0
        raise AssertionError(row)

    # ---------------- Phase A: QKV projection + RoPE + V transpose ----------
    with tc.tile_pool(name="psA", bufs=1, space="PSUM") as psA:
        for j in range(NCH):
            q0 = QCH * j
            cos_sb = stream.tile([128, QCH], F32, tag="cs", bufs=2)
            sin_sb = stream.tile([128, QCH], F32, tag="cs", bufs=2)
            nc.gpsimd.dma_start(out=cos_sb[:], in_=cosT[:, q0 : q0 + QCH])
            nc.gpsimd.dma_start(out=sin_sb[:], in_=sinT[:, q0 : q0 + QCH])

            ps = [
                psA.tile([128, QCH], F32, tag="proj", name=f"proj{j}_{m}", bufs=6)
                for m in range(6)
            ]
            for t in range(HT):
                ht = stream.tile([128, QCH], F32R, tag="ht", bufs=6)
                nc.sync.dma_start(
                    out=ht[:], in_=hT[t * 128 : (t + 1) * 128, q0 : q0 + QCH]
                )
                fl = dict(start=(t == 0), stop=(t == HT - 1), skip_group_check=True)
                for m in range(G):
                    nc.tensor.matmul(
                        ps[m][:],
                        wq_sb[:, t * G * D + m * 128 : t * G * D + (m + 1) * 128],
                        ht[:],
                        **fl,
                    )
                nc.tensor.matmul(
                    ps[4][:], wk_sb[:, t * 128 : (t + 1) * 128], ht[:], **fl
                )
                nc.tensor.matmul(
                    ps[5][:], wv_sb[:, t * 128 : (t + 1) * 128], ht[:], **fl
                )

            # V first (its ACT copy unblocks the PE transposes that sit next
            # in the PE instruction stream), then RoPE for the 4 q heads + k.
            vt = tmp.tile([128, QCH], F32, tag="scratch")
            nc.scalar.copy(vt[:], ps[5][:])
            for b in range(QCH // 128):
                kvi = 4 * j + b
                pst = psA.tile([128, 128], F32, tag="tr", bufs=2)
                nc.tensor.transpose(pst[:], vt[:, b * 128 : (b + 1) * 128], ident[:])
                nc.vector.tensor_copy(v_c[j][:, b * 128 : (b + 1) * 128], pst[:])
                nc.scalar.dma_start(
                    out=v_out[kvi * 128 : (kvi + 1) * 128, :],
                    in_=v_c[j][:, b * 128 : (b + 1) * 128].bitcast(F32),
                )

            # RoPE for the 4 q heads and k: out = ps*cos + rot(ps)*sin_eff
            for m in range(5):
                src = ps[m] if m < G else ps[4]
                if m < G:
                    dst = qT_c[m][j][:]
                else:
                    dst = kT_c[j][:]
                # rotate-half via a PE permutation matmul (a partition-shift
                # SBUF->SBUF DMA here serializes ~2us per copy and delays the
                # PSUM release that gates phase B)
                qraw = tmp.tile([128, QCH], F32R, tag="scratch")
                if m % 2 == 0:
                    nc.scalar.copy(qraw[:], src[:])
                else:
                    nc.vector.tensor_copy(qraw[:], src[:])
                rot_ps = psA.tile([128, QCH], F32, tag="tr", bufs=2)
                nc.tensor.matmul(rot_ps[:], permT_sb[:], qraw[:], start=True, stop=True)
                t2 = tmp.tile([128, QCH], F32, tag="scratch")
                nc.vector.tensor_mul(t2[:], rot_ps[:], sin_sb[:])
                # in-place: qraw *= cos; dst = qraw + t2
                nc.vector.tensor_mul(qraw[:], qraw[:].bitcast(F32), cos_sb[:])
                nc.vector.tensor_add(dst, qraw[:].bitcast(F32), t2[:])
            nc.scalar.dma_start(
                out=kT_out[:, q0 : q0 + QCH], in_=kT_c[j][:].bitcast(F32)
            )

    # wo replaces wq in the shared slot; per-dh slices so o_proj dh=0 starts early
    wo_sb = wbig.tile([128, G * HID], F32R, tag="w")  # [p, (dh n)]
    for dh in range(G):
        nc.gpsimd.dma_start(
            out=wo_sb[:, dh * HID : (dh + 1) * HID],
            in_=woT[dh * 128 : (dh + 1) * 128, :],
        )

    # ---------------- Phase B: attention + o_proj + ReduceScatter -----------
    with tc.tile_pool(name="psB", bufs=1, space="PSUM") as psB:
        for j in range(NCH):
            q0 = QCH * j
            nkv = (q0 + QCH) // 128  # causal: kv tiles 0..nkv-1
            ctxs = [None] * G
            pending = None  # (ctx_ps, den_ps, h) awaiting normalize

            def normalize(ctx_ps, den_ps, h):
                # deferred by one head so the bc matmul (which waits on the
                # DVE reciprocal) doesn't stall the in-order PE stream
                inv = tmp.tile([1, QCH], F32R, tag="inv", bufs=2)
                with nc.allow_low_precision(reason="f32r softmax denom"):
                    nc.vector.reciprocal(inv[:], den_ps[:])
                bc_ps = psB.tile([128, QCH], F32, tag="s", bufs=4)
                nc.tensor.matmul(bc_ps[:], ones_row[:], inv[:], start=True, stop=True)
                ctxc = tmp.tile([128, QCH], F32, tag="scratch")
                nc.scalar.copy(ctxc[:], ctx_ps[:])
                ctx_sb = ctx_pool.tile([128, QCH], F32R, tag="ctx_sb")
                nc.vector.tensor_mul(ctx_sb[:], ctxc[:], bc_ps[:])
                ctxs[h] = ctx_sb

            for h in range(G):
                ctx_ps = psB.tile([128, QCH], F32, tag="ctx", bufs=2)
                den_ps = psB.tile([1, QCH], F32, tag="den", bufs=2)
                for kv in range(nkv):
                    jc, b = kv // 4, kv % 4
                    s_ps = psB.tile([128, QCH], F32, tag="s", bufs=4)
                    nc.tensor.matmul(
                        s_ps[:],
                        kT_c[jc][:, b * 128 : (b + 1) * 128],
                        qT_c[h][j][:],
                        start=True,
                        stop=True,
                    )
                    db = kv - (nkv - 4)
                    if db >= 0:  # diagonal band: apply mask bias
                        nc.vector.tensor_add(
                            s_ps[:], s_ps[:], bias_sb[:, db * QCH : (db + 1) * QCH]
                        )
                    es = es_pool.tile([128, QCH], F32R, tag="es")
                    nc.scalar.activation(
                        es[:], s_ps[:], mybir.ActivationFunctionType.Exp
                    )
                    flk = dict(
                        start=(kv == 0), stop=(kv == nkv - 1), skip_group_check=True
                    )
                    nc.tensor.matmul(
                        ctx_ps[:], v_c[jc][:, b * 128 : (b + 1) * 128], es[:], **flk
                    )
                    nc.tensor.matmul(den_ps[:], ones_col[:], es[:], **flk)
                if pending is not None:
                    normalize(*pending)
                pending = (ctx_ps, den_ps, h)
            normalize(*pending)

            for qt in range(QCH // 128):
                for nh in range(HID // QCH):
                    o_ps = psB.tile([128, QCH], F32, tag="s", bufs=4)
                    for dh in range(G):
                        nc.tensor.matmul(
                            o_ps[:],
                            ctxs[dh][:, qt * 128 : (qt + 1) * 128],
                            wo_sb[:, dh * HID + nh * QCH : dh * HID + (nh + 1) * QCH],
                            start=(dh == 0),
                            stop=(dh == G - 1),
                            skip_group_check=True,
                        )
                    o_sb = osb_pool.tile([128, QCH], ODT, tag="osb", bufs=6)
                    nc.vector.tensor_copy(o_sb[:], o_ps[:])
                    row = q0 + qt * 128
                    pi, pr0 = piece_of_row(row)
                    nc.scalar.dma_start(
                        out=o_parts[pi][
                            row - pr0 : row - pr0 + 128, nh * QCH : (nh + 1) * QCH
                        ],
                        in_=o_sb[:],
                    )
                # fire the RS for any piece whose rows are now fully written
                row_end = q0 + (qt + 1) * 128
                for i, (r0, n) in enumerate(pieces):
                    if r0 + n == row_end:
                        nc.gpsimd.collective_compute(
                            "ReduceScatter",
                            mybir.AluOpType.add,
                            replica_groups=[list(range(N_CORES))],
                            ins=[o_parts[i][:]],
                            outs=[o_shards[i][:]],
                        )
                        # sync queue: a store waiting on the RS would
                        # head-of-line-block the scalar queue's o_part stores
                        nc.sync.dma_start(
                            out=o_out[r0 // N_CORES : (r0 + n) // N_CORES, :],
                            in_=o_shards[i][:],
                        )


def _prep_inputs(hidden_states, attention_mask, cos, sin, Wq, Wk, Wv, Wo):
    h = np.ascontiguousarray(np.asarray(hidden_states, np.float32).reshape(S, HID))
    hT = np.ascontiguousarray(h.T)
    cos2 = np.asarray(cos, np.float32).reshape(S, D)
    sin2 = np.asarray(sin, np.float32).reshape(S, D)
    cosT = np.ascontiguousarray(cos2.T)
    sgn = np.where(np.arange(D) < D // 2, -1.0, 1.0).astype(np.float32)
    sinT = np.ascontiguousarray((sin2 * sgn).T)

    permM = np.zeros((D, D), np.float32)
    for dcol in range(D):
        permM[(dcol + 64) % D, dcol] = 1.0  # lhsT of the rotate-half permutation

    mask2 = np.asarray(attention_mask, np.float32).reshape(S, KV)
    # The kernel hardcodes the causal block structure; verify it holds.
    expect = np.tril(np.ones((S, KV), np.float32))
    if not np.array_equal(mask2, expect):
        raise ValueError("kernel compiled for a causal (tril) attention_mask")
    bias4 = np.empty((4, 128, QCH), np.float32)
    for t in range(4):
        sub = mask2[0:QCH, t * 128 : (t + 1) * 128]  # [q, kv_local]
        bias4[t] = np.where(sub.T > 0.5, 0.0, NEG).astype(np.float32)

    scale = 1.0 / math.sqrt(D)
    Wq = np.asarray(Wq, np.float32)
    Wk = np.asarray(Wk, np.float32)
    Wv = np.asarray(Wv, np.float32)
    Wo = np.asarray(Wo, np.float32)

    in_maps = []
    for c in range(N_CORES):
        wq_c = np.ascontiguousarray((Wq[c * G * D : (c + 1) * G * D, :] * scale).T)
        wk_c = np.ascontiguousarray(Wk[c * D : (c + 1) * D, :].T)
        wv_c = np.ascontiguousarray(Wv[c * D : (c + 1) * D, :].T)
        wo_c = np.ascontiguousarray(Wo[:, c * G * D : (c + 1) * G * D].T)
        in_maps.append(
            dict(
                hT=hT,
                wqT=wq_c,
                wkT=wk_c,
                wvT=wv_c,
                woT=wo_c,
                cosT=cosT,
                sinT=sinT,
                permT=permM,
                bias4=bias4,
            )
        )
    return in_maps


def kernel(
    hidden_states,
    attention_mask,
    cos,
    sin,
    past_key,
    past_value,
    Wq,
    Wk,
    Wv,
    Wo,
    seq_positions,
    batch_position,
):
    global _compiled, LAST_RESULT
    assert int(np.asarray(seq_positions).reshape(-1)[0]) == 0
    assert int(np.asarray(batch_position)) == 0

    if _compiled is None:
        _compiled = _build()
    nc = _compiled

    in_maps = _prep_inputs(hidden_states, attention_mask, cos, sin, Wq, Wk, Wv, Wo)
    res = run_bass_kernel_spmd(nc, in_maps, list(range(N_CORES)), trace=TRACE)
    LAST_RESULT = res

    pieces = PIECES
    key_cache = np.empty((1, KVH, KV, D), np.float32)
    value_cache = np.empty((1, KVH, KV, D), np.float32)
    attn_out = np.empty((S, HID), np.float32)
    for c in range(N_CORES):
        r = res.results[c]
        key_cache[0, c] = r["kT_out"].T
        value_cache[0, c] = r["v_out"]
        o_np = np.asarray(r["o_out"], np.float32)
        for r0, n in pieces:
            sh = n // N_CORES
            attn_out[r0 + sh * c : r0 + sh * (c + 1)] = o_np[
                r0 // N_CORES : r0 // N_CORES + sh
            ]
    return attn_out.reshape(1, S, HID), key_cache, value_cache
